# revision 1
# baseline (speedup 1.0000x reference)
# Trainium2 Bass kernel for single-head causal attention
#   q = x@Wq, k = x@Wk, v = x@Wv   (x [B,T,C], W* [C,H])
#   out = softmax(mask(q k^T / sqrt(C))) @ v
# B=512, T=142, C=512, H=64.  Data-parallel over B across 8 NeuronCores.
#
# Device-side layout strategy (per core, 64 batches = 9088 tokens):
#  - host feeds x^T  [4,128,9088]  (contraction dim C on partitions)
#  - qT = Wq-stationary matmuls -> psum [128,*] rows 0:64 (Wq zero-padded)
#  - k,v packed:  [Wk|Wv] stationary -> psum rows 0:64 = kT, 64:128 = vT
#  - scores weiT[s,t] = kT-stationary matmul; causal mask added via one
#    identity-stationary matmul accumulating a mask tile into PSUM
#  - exp on ScalarE (scale=C^-0.5 fused), result bf16 in SBUF
#  - v natural [s,h] via identity-matmul transpose of vT
#  - AV: exp-scores stationary, rhs = [v | ones] -> out [t, 65] where
#    col 64 = softmax denominator; division happens on host (glue).
# Groups of 3 batches; group PAIRS share one x DMA and one output DMA
# to keep the SP sequencer / HWDGE ring off the critical path.
import os

import numpy as np
import ml_dtypes

B, T, C, H = 512, 142, 512, 64
NCORES = 8
NB = B // NCORES            # 64 batches per core
NT = NB * T                 # 9088 tokens per core
GB = 3                      # batches per processing group
NG = (NB + GB - 1) // GB    # 22 groups (21 full + 1 single)
SCALE = float(C) ** -0.5
NEG = -1e30
TW = 65                     # out block width: H + 1 denominator column

_CACHE = {}


def _groups():
    return [(g * GB, min(GB, NB - g * GB)) for g in range(NG)]


def _build_nc():
    import concourse.bacc as bacc
    import concourse.mybir as mybir
    from concourse.tile import TileContext

    fp32 = mybir.dt.float32
    bf16 = mybir.dt.bfloat16
    Exp = mybir.ActivationFunctionType.Exp

    nc = bacc.Bacc(
        "TRN2",
        target_bir_lowering=False,
        debug=False,
        enable_asserts=False,
        num_devices=NCORES,
    )

    xt = nc.dram_tensor("xt", [4, 128, NT], bf16, kind="ExternalInput").ap()
    # all 8 weight chunks in one tensor: [Wq|0] chunks then [Wk|Wv] chunks
    wts = nc.dram_tensor("wts", [8, 128, 128], bf16, kind="ExternalInput").ap()
    # constants blob: cols 0:426 mask3, 426:468 mskt3 (rows 0:14),
    # 468:596 identity128, 596:660 idhi
    cst = nc.dram_tensor("cst", [128, 660], bf16, kind="ExternalInput").ap()
    om = nc.dram_tensor("om", [NG, 128, GB * TW], fp32, kind="ExternalOutput").ap()
    ot = nc.dram_tensor("ot", [NG, 14, GB * TW], fp32, kind="ExternalOutput").ap()

    GT = GB * T           # 426 token columns per full group
    TAIL0 = GT            # col offset of tail score blocks in psc
    groups = _groups()
    pairs = [(2 * p, min(2, NG - 2 * p)) for p in range((NG + 1) // 2)]

    with TileContext(nc) as tc:
        with (
            tc.tile_pool(name="const", bufs=1) as cpool,
            tc.tile_pool(name="xtp", bufs=2) as xpool,
            tc.tile_pool(name="work", bufs=3) as wpool,
            tc.tile_pool(name="psum", bufs=1, space="PSUM") as ppool,
        ):
            wts_sb = cpool.tile([128, 8 * 128], bf16)
            cst_sb = cpool.tile([128, 660], bf16)
            nc.sync.dma_start(
                out=wts_sb.rearrange("p (c w) -> p c w", c=8),
                in_=wts.rearrange("c p w -> p c w"))
            nc.sync.dma_start(out=cst_sb[:, :], in_=cst)

            def wq_c(c):
                return wts_sb[:, c * 128:(c + 1) * 128]

            def wkv_c(c):
                return wts_sb[:, 512 + c * 128:512 + (c + 1) * 128]

            msk3_sb = cst_sb[:, 0:426]
            mskt3_sb = cst_sb[0:14, 426:468]
            iden_sb = cst_sb[:, 468:596]
            idhi_sb = cst_sb[:, 596:660]

            for g0, np_ in pairs:
                pg = groups[g0:g0 + np_]
                gtp = sum(nb for _, nb in pg) * T
                t0 = pg[0][0] * T

                xt_t = xpool.tile([128, 4 * 2 * GT], bf16, tag="xt")
                if g0 == 0:
                    for c in range(4):
                        nc.sync.dma_start(
                            out=xt_t[:, c * gtp:(c + 1) * gtp],
                            in_=xt[c, :, t0:t0 + gtp],
                        )
                else:
                    nc.sync.dma_start(
                        out=xt_t[:, 0:4 * gtp].rearrange("p (c t) -> p c t", c=4),
                        in_=xt[:, :, t0:t0 + gtp].rearrange("c p t -> p c t"),
                    )

                o_sb = wpool.tile([128, 2 * GB * TW], fp32, tag="o")
                o2_sb = wpool.tile([14, 2 * GB * TW], fp32, tag="o2")

                for s, (b0, nb) in enumerate(pg):
                    gt = nb * T
                    off = (b0 * T) - t0          # token offset within pair tile

                    # ---- QKV projections ----
                    pq = ppool.tile([128, GT], fp32, tag="pq", bufs=2)
                    pkv = ppool.tile([128, GT], fp32, tag="pkv", bufs=2)
                    # kv first: the ACT kv-copy (scores' stationary operand)
                    # then overlaps the q matmuls on PE
                    for c in range(4):
                        rhs = xt_t[:, c * gtp + off:c * gtp + off + gt]
                        nc.tensor.matmul(
                            pkv[:, :gt], lhsT=wkv_c(c), rhs=rhs,
                            start=(c == 0), stop=(c == 3),
                        )
                    q_sb = wpool.tile([64, GT], bf16, tag="q")
                    kv_sb = wpool.tile([128, GT], bf16, tag="kv")
                    nc.scalar.copy(kv_sb[:, :gt], pkv[:, :gt])
                    for c in range(4):
                        rhs = xt_t[:, c * gtp + off:c * gtp + off + gt]
                        nc.tensor.matmul(
                            pq[:, :gt], lhsT=wq_c(c), rhs=rhs,
                            start=(c == 0), stop=(c == 3),
                        )
                    nc.vector.tensor_copy(q_sb[:, :gt], pq[0:64, :gt])

                    # ---- scores weiT[s,t] + causal mask ----
                    psc = ppool.tile([128, GT + GB * 14], fp32, tag="psc", bufs=2)
                    for j in range(nb):
                        cl = j * T
                        nc.tensor.matmul(
                            psc[:, cl:cl + T],
                            lhsT=kv_sb[0:64, cl:cl + 128],
                            rhs=q_sb[0:64, cl:cl + T],
                            start=True, stop=False,
                        )
                        nc.tensor.matmul(
                            psc[:, cl:cl + T],
                            lhsT=iden_sb,
                            rhs=msk3_sb[:, 0:T],
                            start=False, stop=True,
                        )
                        tco = TAIL0 + j * 14
                        nc.tensor.matmul(
                            psc[0:14, tco:tco + 14],
                            lhsT=kv_sb[0:64, cl + 128:cl + T],
                            rhs=q_sb[0:64, cl + 128:cl + T],
                            start=True, stop=False,
                        )
                        nc.tensor.matmul(
                            psc[0:14, tco:tco + 14],
                            lhsT=iden_sb[0:14, 0:14],
                            rhs=mskt3_sb[:, 0:14],
                            start=False, stop=True,
                        )

                    exp_sb = wpool.tile([128, GT + GB * 14], bf16, tag="exp")
                    nc.scalar.activation(
                        exp_sb[:, 0:gt], psc[:, 0:gt], Exp, scale=SCALE)
                    nc.scalar.activation(
                        exp_sb[0:14, TAIL0:TAIL0 + nb * 14],
                        psc[0:14, TAIL0:TAIL0 + nb * 14],
                        Exp, scale=SCALE,
                    )

                    # ---- v natural via identity-matmul transpose ----
                    pvt = ppool.tile([128, GB * 128], fp32, tag="pvt")
                    for j in range(nb):
                        cl = j * T
                        nc.tensor.matmul(
                            pvt[:, j * 64:(j + 1) * 64],
                            lhsT=kv_sb[64:128, cl:cl + 128],
                            rhs=idhi_sb[64:128, :],
                            start=True, stop=True,
                        )
                        nc.tensor.matmul(
                            pvt[0:14, GB * 64 + j * 64:GB * 64 + (j + 1) * 64],
                            lhsT=kv_sb[64:128, cl + 128:cl + T],
                            rhs=idhi_sb[64:128, :],
                            start=True, stop=True,
                        )
                    vex_sb = wpool.tile([128, GB * TW], bf16, tag="vex")
                    vext_sb = wpool.tile([14, GB * TW], bf16, tag="vext")
                    nc.vector.tensor_copy(
                        vex_sb.rearrange("p (b h) -> p b h", h=TW)[:, 0:nb, 0:64],
                        pvt[:, 0:nb * 64].rearrange("p (b h) -> p b h", h=64),
                    )
                    nc.vector.tensor_copy(
                        vext_sb.rearrange("p (b h) -> p b h", h=TW)[:, 0:nb, 0:64],
                        pvt[0:14, GB * 64:GB * 64 + nb * 64].rearrange(
                            "p (b h) -> p b h", h=64),
                    )
                    nc.vector.memset(
                        vex_sb.rearrange("p (b h) -> p b h", h=TW)[:, 0:nb, 64:65],
                        1.0)
                    nc.vector.memset(
                        vext_sb.rearrange("p (b h) -> p b h", h=TW)[:, 0:nb, 64:65],
                        1.0)

                    # ---- AV: out[t,0:64] = sum_s P^T[s,t] v[s,:], col64=denom ----
                    pout = ppool.tile([128, 2 * GB * TW], fp32, tag="pout")
                    TL = GB * TW
                    for j in range(nb):
                        cl = j * T
                        nc.tensor.matmul(
                            pout[:, j * TW:(j + 1) * TW],
                            lhsT=exp_sb[:, cl:cl + 128],
                            rhs=vex_sb[:, j * TW:(j + 1) * TW],
                            start=True, stop=True,
                        )
                        nc.tensor.matmul(
                            pout[0:14, TL + j * TW:TL + (j + 1) * TW],
                            lhsT=exp_sb[:, cl + 128:cl + T],
                            rhs=vex_sb[:, j * TW:(j + 1) * TW],
                            start=True, stop=False,
                        )
                        nc.tensor.matmul(
                            pout[0:14, TL + j * TW:TL + (j + 1) * TW],
                            lhsT=exp_sb[0:14, TAIL0 + j * 14:TAIL0 + (j + 1) * 14],
                            rhs=vext_sb[0:14, j * TW:(j + 1) * TW],
                            start=False, stop=True,
                        )

                    oc = s * GB * TW
                    nc.scalar.copy(
                        o_sb[:, oc:oc + nb * TW], pout[:, 0:nb * TW])
                    nc.vector.tensor_copy(
                        o2_sb[0:14, oc:oc + nb * TW],
                        pout[0:14, TL:TL + nb * TW])

                # ---- batched output stores (one per pair per tensor) ----
                last_nb = pg[-1][1]
                if np_ == 2 and last_nb == GB:
                    nc.gpsimd.dma_start(
                        out=om[g0:g0 + 2].rearrange("g p c -> p g c"),
                        in_=o_sb.rearrange("p (g c) -> p g c", g=2),
                    )
                    nc.gpsimd.dma_start(
                        out=ot[g0:g0 + 2].rearrange("g p c -> p g c"),
                        in_=o2_sb.rearrange("p (g c) -> p g c", g=2),
                    )
                else:
                    for s, (b0, nb) in enumerate(pg):
                        oc = s * GB * TW
                        nc.gpsimd.dma_start(
                            out=om[g0 + s, :, 0:nb * TW],
                            in_=o_sb[:, oc:oc + nb * TW])
                        nc.gpsimd.dma_start(
                            out=ot[g0 + s, :, 0:nb * TW],
                            in_=o2_sb[0:14, oc:oc + nb * TW])

    nc.compile()
    return nc


def _prep_shared(Wq, Wk, Wv):
    bf16 = ml_dtypes.bfloat16
    wq_pad = np.concatenate([Wq, np.zeros((C, H), np.float32)], axis=1)
    wkv = np.concatenate([Wk, Wv], axis=1)
    wts_np = np.concatenate(
        [
            np.ascontiguousarray(wq_pad.reshape(4, 128, 128)),
            np.ascontiguousarray(wkv.reshape(4, 128, 128)),
        ],
        axis=0,
    ).astype(bf16)

    s = np.arange(128)[:, None]
    t = np.arange(T)[None, :]
    msk = np.where(s <= t, 0.0, NEG).astype(np.float32)
    i = np.arange(14)[:, None]
    j = np.arange(14)[None, :]
    mskt = np.where(i <= j, 0.0, NEG).astype(np.float32)
    idhi = np.zeros((128, 64), np.float32)
    idhi[64 + np.arange(64), np.arange(64)] = 1.0

    cst = np.zeros((128, 660), np.float32)
    cst[:, 0:426] = np.tile(msk, (1, 3))
    cst[0:14, 426:468] = np.tile(mskt, (1, 3))
    cst[:, 468:596] = np.eye(128, dtype=np.float32)
    cst[:, 596:660] = idhi
    return dict(wts=wts_np, cst=cst.astype(bf16))


def _prep_core_xt(x_core):
    # x_core [NB, T, C] fp32 -> [4, 128, NT] bf16 (x^T, C on partitions)
    xt = x_core.reshape(NT, C).T            # [C, NT] view
    xt = np.ascontiguousarray(xt).reshape(4, 128, NT)
    return xt.astype(ml_dtypes.bfloat16)


def _assemble_core(om_np, ot_np):
    # om [NG, 128, GB*TW], ot [NG, 14, GB*TW] -> [NB, T, H] normalized
    bm = om_np.reshape(NG, 128, GB, TW).transpose(0, 2, 1, 3).reshape(NG * GB, 128, TW)
    bt = ot_np.reshape(NG, 14, GB, TW).transpose(0, 2, 1, 3).reshape(NG * GB, 14, TW)
    bm = bm[:NB].astype(np.float32)
    bt = bt[:NB].astype(np.float32)
    full = np.concatenate([bm, bt], axis=1)         # [NB, 142, TW]
    return full[:, :, 0:H] / full[:, :, H:H + 1]


def kernel(**inputs):
    x = np.asarray(inputs["x"], dtype=np.float32)
    Wq = np.asarray(inputs["Wq"], dtype=np.float32)
    Wk = np.asarray(inputs["Wk"], dtype=np.float32)
    Wv = np.asarray(inputs["Wv"], dtype=np.float32)

    from concourse.bass_utils import run_bass_kernel_spmd

    if "nc" not in _CACHE:
        _CACHE["nc"] = _build_nc()
    nc = _CACHE["nc"]

    shared = _prep_shared(Wq, Wk, Wv)
    in_maps = []
    for core in range(NCORES):
        m = dict(shared)
        m["xt"] = _prep_core_xt(x[core * NB:(core + 1) * NB])
        in_maps.append(m)

    trace = bool(int(os.environ.get("TRN_KERNEL_TRACE", "0")))
    res = run_bass_kernel_spmd(
        nc, in_maps, core_ids=list(range(NCORES)), trace=trace,
    )
    _CACHE["last_result"] = res

    outs = []
    for core in range(NCORES):
        r = res.results[core]
        outs.append(_assemble_core(np.asarray(r["om"]), np.asarray(r["ot"])))
    return np.concatenate(outs, axis=0).astype(np.float32)



# revision 43
# speedup vs baseline: 1.3921x; 1.3921x over previous
# Trainium2 Bass kernel for single-head causal attention
#   q = x@Wq, k = x@Wk, v = x@Wv   (x [B,T,C], W* [C,H])
#   out = softmax(mask(q k^T / sqrt(C))) @ v
# B=512, T=142, C=512, H=64.  Data-parallel over B across 8 NeuronCores.
#
# Per-core layout (64 batches, 21 groups of 3 + 1 tail group):
#  - host reorders tokens per group: [b0 0:128 | b1 0:128 | b2 0:128 |
#    b0 128:142 | b1 128:142 | b2 128:142] so every PE operand is a
#    contiguous column range.
#  - [Wq|Wk] packed stationary -> one PSUM accumulation; qT rows 0:64,
#    kT rows 64:128; a partition-shifted copy re-bases k to rows 0:64.
#  - v computed x-stationary directly in natural [token, h] layout;
#    the 3 batch tails share one 42-column stationary tile; all v tiles
#    leave PSUM in a single engine copy.
#  - causal mask applied as a 0/1 bf16 multiply on DVE after exp
#    (no mask matmuls on the PE); main-keys-x-tail-queries columns are
#    fully valid and skip the mask entirely.
#  - tail scores use all 3 batches' key-tails as one stationary; the
#    cross-batch terms are zeroed by the mask multiply, enabling ONE
#    fused tail-AV matmul per group.
#  - AV is v-stationary [v|1] (65 cols) -> out^T [65, t] with the
#    softmax denominator in row 64; division happens on host.
#  - 3-iteration software pipeline: scores lag 1, AV lag 3 behind the
#    projections, so exp/mask latency never stalls the PE.
import os

import numpy as np
import ml_dtypes

B, T, C, H = 512, 142, 512, 64
NCORES = 8
NB = B // NCORES            # 64 batches per core
NT = NB * T                 # 9088 tokens per core
GB = 3                      # batches per full group
NG = 22                     # 21 full groups + 1 single-batch group
NFULL = 21
SCALE = float(C) ** -0.5
TW = 65                     # out rows: H + denominator row
OSPLIT = 284                # o-copy column split between ACT and DVE

# wcst column layout (all bf16)
WQK0 = 0                    # 4 chunks of [Wq_c | Wk_c] at 128*c
WV0 = 512                   # 4 chunks of Wv_c at 512 + 64*c
MSKF0 = 768                 # full-group mask [128, 426]
MSKT0 = 1194                # tail-group mask [128, 142]
WCST_COLS = 1336

_CACHE = {}


def _group_info(g):
    if g < NFULL:
        return GB, g * (GB * T)
    return 1, NFULL * (GB * T)


def _pairs():
    # pair p covers groups (2p, 2p+1); last pair = (20, 21)
    return [(p, 2 * p, min(2, NG - 2 * p)) for p in range((NG + 1) // 2)]


def _build_nc():
    import concourse.bacc as bacc
    import concourse.mybir as mybir
    from concourse.tile import TileContext

    fp32 = mybir.dt.float32
    fp16 = mybir.dt.float16
    bf16 = mybir.dt.bfloat16
    Exp = mybir.ActivationFunctionType.Exp

    nc = bacc.Bacc(
        "TRN2",
        target_bir_lowering=False,
        debug=False,
        enable_asserts=False,
        num_devices=NCORES,
    )

    xt = nc.dram_tensor("xt", [4, 128, NT], bf16, kind="ExternalInput").ap()
    # weights and masks staged separately so the critical first transfer
    # (wqkv, SP queue) is small; masks follow on the ACT queue.
    wqkv = nc.dram_tensor("wqkv", [128, 768], bf16, kind="ExternalInput").ap()
    msk = nc.dram_tensor("msk", [128, WCST_COLS - 768], bf16,
                         kind="ExternalInput").ap()
    om = nc.dram_tensor("om", [NG, TW, 468], fp16,
                        kind="ExternalOutput").ap()

    with TileContext(nc) as tc:
        with (
            tc.tile_pool(name="const", bufs=1) as cpool,
            tc.tile_pool(name="xtp", bufs=6) as xpool,
            tc.tile_pool(name="work", bufs=3) as wpool,
            tc.tile_pool(name="psum", bufs=1, space="PSUM") as ppool,
        ):
            wcst_sb = cpool.tile([128, WCST_COLS], bf16)
            # SWDGE path: its descriptor generation (Pool) runs in parallel
            # with the HWDGE generation of the first x transfers
            nc.gpsimd.dma_start(out=wcst_sb[:, 0:768], in_=wqkv)

            def wqk_c(c):
                return wcst_sb[:, WQK0 + c * 128:WQK0 + (c + 1) * 128]

            def wv_c(c):
                return wcst_sb[:, WV0 + c * 64:WV0 + (c + 1) * 64]

            xt_tiles = {}
            qsb, ksb, psc_t, exp_t, expm_t, vex_t = {}, {}, {}, {}, {}, {}
            pout_t = {}

            def issue_x_dma(g):
                nb, t0 = _group_info(g)
                gt = nb * T
                xt_t = xpool.tile([128, 4 * GB * T], bf16, tag="xt")
                xt_tiles[g] = xt_t
                if g == 0:
                    # split in halves for the first group so PE starts early
                    # (each extra DMA costs 625ns of serialized HWDGE gen)
                    for h in range(2):
                        nc.sync.dma_start(
                            out=xt_t[:, 2 * h * gt:2 * (h + 1) * gt].rearrange(
                                "p (c t) -> p c t", c=2),
                            in_=xt[2 * h:2 * (h + 1), :, 0:gt].rearrange(
                                "c p t -> p c t"))
                else:
                    nc.sync.dma_start(
                        out=xt_t[:, 0:4 * gt].rearrange(
                            "p (c t) -> p c t", c=4),
                        in_=xt[:, :, t0:t0 + gt].rearrange("c p t -> p c t"))

            def xslice(g, c, lo, hi):
                gt = _group_info(g)[0] * T
                base = c * gt
                return xt_tiles[g][:, base + lo:base + hi]

            def emit_qk(g):
                nb, _ = _group_info(g)
                gt = nb * T
                pqk = ppool.tile([128, GB * T], fp32, tag="pqk", bufs=2)
                for c in range(4):
                    nc.tensor.matmul(
                        pqk[:, 0:gt], lhsT=wqk_c(c), rhs=xslice(g, c, 0, gt),
                        start=(c == 0), stop=(c == 3))
                # one full-width PSUM->SBUF copy; rows 0:64 are qT, usable
                # directly.  kT (rows 64:128) is re-based to partition 0 by
                # a cheap SBUF->SBUF DMA so the scores operands share a base.
                qk_sb = wpool.tile([128, GB * T], bf16, tag="qk")
                k_sb = wpool.tile([64, GB * T], bf16, tag="k2")
                nc.scalar.copy(qk_sb[:, 0:gt], pqk[:, 0:gt])
                if 2 <= g < NG - 2:
                    # steady state: re-base kT via SBUF->SBUF DMA (off the
                    # compute engines; latency hidden by the 2-deep lag)
                    nc.sync.dma_start(
                        out=k_sb[0:64, 0:gt], in_=qk_sb[64:128, 0:gt])
                elif g < 2:
                    # warm-up: ACT is idle and the DMA round trip (+900ns
                    # sem) would sit on the critical path
                    nc.scalar.copy(k_sb[0:64, 0:gt], pqk[64:128, 0:gt])
                else:
                    # drain: keep the re-base on ACT, off the DVE o-chain
                    nc.scalar.copy(k_sb[0:64, 0:gt], pqk[64:128, 0:gt])
                qsb[g], ksb[g] = qk_sb, k_sb

            def emit_v(g):
                nb, _ = _group_info(g)
                pvt = ppool.tile([128, 256], fp32, tag="pvt", bufs=3)
                for j in range(nb):
                    for c in range(4):
                        nc.tensor.matmul(
                            pvt[:, j * 64:(j + 1) * 64],
                            lhsT=xslice(g, c, j * 128, (j + 1) * 128),
                            rhs=wv_c(c), start=(c == 0), stop=(c == 3))
                for c in range(4):
                    nc.tensor.matmul(
                        pvt[0:nb * 14, nb * 64:nb * 64 + 64],
                        lhsT=xslice(g, c, nb * 128, nb * 128 + nb * 14),
                        rhs=wv_c(c), start=(c == 0), stop=(c == 3))
                # one copy moves main v tiles AND the packed tail tile
                # (rows 42:126 of the tail block are dead but harmless)
                vex = wpool.tile([128, 4 * TW], bf16, tag="vex", bufs=5)
                nbl = nb + 1
                nc.vector.tensor_copy(
                    vex.rearrange("p (b h) -> p b h", h=TW)[:, 0:nbl, 0:64],
                    pvt[:, 0:nbl * 64].rearrange("p (b h) -> p b h", h=64),
                )
                if g < 5:  # ones columns: written once per pool buffer
                    nc.vector.memset(
                        vex.rearrange("p (b h) -> p b h", h=TW)[:, :, 64:65],
                        1.0)
                vex_t[g] = vex

            def emit_scores(g):
                # psc columns: [main nb*128 | sctt nb*14 | m2 nb*14]
                nb, _ = _group_info(g)
                MAIN = nb * 128
                S0, M0 = MAIN, MAIN + nb * 14
                psc = ppool.tile([128, 468], fp32, tag="psc", bufs=2)
                psc_t[g] = psc
                if g < 2:
                    # NaN-proof the never-written sctt rows once per buffer
                    # (engine partition windows: base 32 allows <=32 rows,
                    # base 64 allows <=64; rows 32:42 are rewritten by the
                    # tail-score matmuls right after)
                    nc.vector.memset(psc[32:64, S0:M0], 0.0)
                    nc.vector.memset(psc[64:128, S0:M0], 0.0)
                q_sb, k_sb = qsb.pop(g), ksb.pop(g)
                for j in range(nb):
                    nc.tensor.matmul(
                        psc[:, j * 128:(j + 1) * 128],
                        lhsT=k_sb[0:64, j * 128:(j + 1) * 128],
                        rhs=q_sb[0:64, j * 128:(j + 1) * 128],
                        start=True, stop=True)
                    nc.tensor.matmul(
                        psc[:, M0 + j * 14:M0 + (j + 1) * 14],
                        lhsT=k_sb[0:64, j * 128:(j + 1) * 128],
                        rhs=q_sb[0:64, MAIN + j * 14:MAIN + (j + 1) * 14],
                        start=True, stop=True)
                    nc.tensor.matmul(
                        psc[0:nb * 14, S0 + j * 14:S0 + (j + 1) * 14],
                        lhsT=k_sb[0:64, MAIN:MAIN + nb * 14],
                        rhs=q_sb[0:64, MAIN + j * 14:MAIN + (j + 1) * 14],
                        start=True, stop=True)

            def emit_exp(g):
                nb, _ = _group_info(g)
                expc = nb * 156
                mw = nb * 142        # masked width: main + sctt
                psc = psc_t.pop(g)
                exp_sb = wpool.tile([128, 468], bf16, tag="exp", bufs=3)
                expm = wpool.tile([128, 426], bf16, tag="expm", bufs=3)
                nc.scalar.activation(
                    exp_sb[:, 0:expc], psc[:, 0:expc], Exp, scale=SCALE)
                mo = MSKF0 if nb == GB else MSKT0
                nc.vector.tensor_mul(
                    expm[:, 0:mw], exp_sb[:, 0:mw], wcst_sb[:, mo:mo + mw])
                exp_t[g], expm_t[g] = exp_sb, expm

            def emit_av(g):
                nb, _ = _group_info(g)
                MAIN = nb * 128
                S0, M0 = MAIN, MAIN + nb * 14
                exp_sb, expm = exp_t.pop(g), expm_t.pop(g)
                vex = vex_t.pop(g)
                pout = ppool.tile([TW, 468], fp32, tag="pout", bufs=1)
                pout_t[g] = pout
                for j in range(nb):
                    nc.tensor.matmul(
                        pout[0:TW, j * 128:(j + 1) * 128],
                        lhsT=vex[:, j * TW:(j + 1) * TW],
                        rhs=expm[0:128, j * 128:(j + 1) * 128],
                        start=True, stop=True)
                for j in range(nb):
                    nc.tensor.matmul(
                        pout[0:TW, MAIN + j * 14:MAIN + (j + 1) * 14],
                        lhsT=vex[:, j * TW:(j + 1) * TW],
                        rhs=exp_sb[0:128, M0 + j * 14:M0 + (j + 1) * 14],
                        start=True, stop=True)
                # tail-key contributions land in their own columns
                # (M0:M0+nb*14); the host adds the two partial sums
                nc.tensor.matmul(
                    pout[0:TW, M0:M0 + nb * 14],
                    lhsT=vex[0:nb * 14, nb * TW:(nb + 1) * TW],
                    rhs=expm[0:nb * 14, S0:M0],
                    start=True, stop=True)

            def emit_out(g):
                nb, _ = _group_info(g)
                ow = nb * 156
                pout = pout_t.pop(g)
                o_sb = wpool.tile([TW, 468], fp16, tag="o", bufs=3)
                nc.vector.tensor_copy(o_sb[0:TW, 0:ow], pout[0:TW, 0:ow])
                eng = nc.sync if g == NG - 1 else nc.gpsimd
                eng.dma_start(out=om[g, :, 0:ow], in_=o_sb[0:TW, 0:ow])

            # software pipeline: iteration i runs
            #   qk(i), AV(i-4), out(i-4), scores(i-2), exp/mask(i-2), v(i)
            # with compressed lags over the last groups (drain phase: the
            # PE is idle there, so latency-hiding lags only stretch the
            # tail)
            sc_at = {g: g + 2 for g in range(NG)}
            av_at = {g: g + 4 for g in range(NG)}
            last_i = max(av_at.values())
            for g in range(3):
                issue_x_dma(g)
            for i in range(last_i + 1):
                if i + 3 < NG:
                    issue_x_dma(i + 3)
                if i == 1:
                    # masks are first read by mask(0) at iteration 2;
                    # deferring this transfer keeps the early DMA queue
                    # clear for x chunks and the first k re-base DMAs
                    nc.scalar.dma_start(
                        out=wcst_sb[:, 768:WCST_COLS], in_=msk)
                if i < NG:
                    emit_qk(i)
                for g in range(NG):
                    if av_at[g] == i:
                        emit_av(g)
                        emit_out(g)
                for g in range(NG):
                    if sc_at[g] == i:
                        emit_scores(g)
                        emit_exp(g)
                if i < NG:
                    emit_v(i)

    nc.compile()
    return nc


def _prep_shared(Wq, Wk, Wv):
    bf16 = ml_dtypes.bfloat16
    wqkv = np.zeros((128, 768), np.float32)
    for c in range(4):
        wqkv[:, WQK0 + c * 128:WQK0 + c * 128 + 64] = Wq[c * 128:(c + 1) * 128]
        wqkv[:, WQK0 + c * 128 + 64:WQK0 + (c + 1) * 128] = \
            Wk[c * 128:(c + 1) * 128]
        wqkv[:, WV0 + c * 64:WV0 + (c + 1) * 64] = Wv[c * 128:(c + 1) * 128]

    s = np.arange(128)[:, None]
    t = np.arange(128)[None, :]
    tri128 = (s <= t).astype(np.float32)          # valid (unmasked) = 1
    s14 = np.arange(14)[:, None]
    t14 = np.arange(14)[None, :]
    tri14 = (s14 <= t14).astype(np.float32)

    msk = np.zeros((128, WCST_COLS - 768), np.float32)
    mf = MSKF0 - 768
    for j in range(3):
        msk[:, mf + j * 128:mf + (j + 1) * 128] = tri128
    for j in range(3):
        msk[14 * j:14 * (j + 1), mf + 384 + 14 * j:mf + 384 + 14 * (j + 1)] \
            = tri14
    mt = MSKT0 - 768
    msk[:, mt:mt + 128] = tri128
    msk[0:14, mt + 128:mt + 142] = tri14
    return dict(wqkv=wqkv.astype(bf16), msk=msk.astype(bf16))


def _perm():
    # reordered token index -> natural (b*T + t) index, per core
    idx = []
    for g in range(NFULL):
        for j in range(GB):
            b = GB * g + j
            idx.append(np.arange(b * T, b * T + 128))
        for j in range(GB):
            b = GB * g + j
            idx.append(np.arange(b * T + 128, (b + 1) * T))
    b = NB - 1
    idx.append(np.arange(b * T, b * T + 128))
    idx.append(np.arange(b * T + 128, (b + 1) * T))
    return np.concatenate(idx)


_PERM = _perm()


def _prep_core_xt(x_core):
    # x_core [NB, T, C] fp32 -> [4, 128, NT] bf16 (x^T, reordered tokens)
    xall = x_core.reshape(NT, C)[_PERM]
    xtr = np.ascontiguousarray(xall.T).reshape(4, 128, NT)
    return xtr.astype(ml_dtypes.bfloat16)


def _assemble_core(om_np):
    # om [NG, 65, 468] f16 -> [NB, T, H] normalized fp32
    # tail-query columns hold two partial sums (main keys at 384:426,
    # tail keys at 426:468) that are added here
    om_np = om_np.astype(np.float32)
    full = om_np[:NFULL]                                 # [21, 65, 468]
    mains = full[:, :, 0:384].reshape(NFULL, TW, 3, 128)
    mains = mains.transpose(0, 2, 3, 1).reshape(63, 128, TW)
    tails = (full[:, :, 384:426] + full[:, :, 426:468]).reshape(
        NFULL, TW, 3, 14)
    tails = tails.transpose(0, 2, 3, 1).reshape(63, 14, TW)
    toks = np.concatenate([mains, tails], axis=1)        # [63, 142, 65]
    lt = om_np[NFULL, :, 128:142] + om_np[NFULL, :, 142:156]
    last = np.concatenate(
        [om_np[NFULL, :, 0:128].T, lt.T], axis=0)[None]  # [1, 142, 65]
    allb = np.concatenate([toks, last], axis=0)          # [64, 142, 65]
    return allb[:, :, 0:H] / allb[:, :, H:H + 1]


def kernel(**inputs):
    x = np.asarray(inputs["x"], dtype=np.float32)
    Wq = np.asarray(inputs["Wq"], dtype=np.float32)
    Wk = np.asarray(inputs["Wk"], dtype=np.float32)
    Wv = np.asarray(inputs["Wv"], dtype=np.float32)

    from concourse.bass_utils import run_bass_kernel_spmd

    if "nc" not in _CACHE:
        _CACHE["nc"] = _build_nc()
    nc = _CACHE["nc"]

    shared = _prep_shared(Wq, Wk, Wv)
    in_maps = []
    for core in range(NCORES):
        m = dict(shared)
        m["xt"] = _prep_core_xt(x[core * NB:(core + 1) * NB])
        in_maps.append(m)

    trace = bool(int(os.environ.get("TRN_KERNEL_TRACE", "0")))
    res = run_bass_kernel_spmd(
        nc, in_maps, core_ids=list(range(NCORES)), trace=trace,
    )
    _CACHE["last_result"] = res

    outs = []
    for core in range(NCORES):
        r = res.results[core]
        outs.append(_assemble_core(np.asarray(r["om"])))
    return np.concatenate(outs, axis=0).astype(np.float32)


# revision 56
# speedup vs baseline: 1.4010x; 1.0064x over previous
# Trainium2 Bass kernel for single-head causal attention
#   q = x@Wq, k = x@Wk, v = x@Wv   (x [B,T,C], W* [C,H])
#   out = softmax(mask(q k^T / sqrt(C))) @ v
# B=512, T=142, C=512, H=64.  Data-parallel over B across 8 NeuronCores.
#
# Per-core layout (64 batches, 21 groups of 3 + 1 tail group):
#  - host reorders tokens per group: [b0 0:128 | b1 0:128 | b2 0:128 |
#    b0 128:142 | b1 128:142 | b2 128:142] so every PE operand is a
#    contiguous column range.
#  - [Wq|Wk] packed stationary -> one PSUM accumulation; qT rows 0:64,
#    kT rows 64:128; a partition-shifted copy re-bases k to rows 0:64.
#  - v computed x-stationary directly in natural [token, h] layout;
#    the 3 batch tails share one 42-column stationary tile; all v tiles
#    leave PSUM in a single engine copy.
#  - causal mask applied as a 0/1 bf16 multiply on DVE after exp
#    (no mask matmuls on the PE); main-keys-x-tail-queries columns are
#    fully valid and skip the mask entirely.
#  - tail scores use all 3 batches' key-tails as one stationary; the
#    cross-batch terms are zeroed by the mask multiply, enabling ONE
#    fused tail-AV matmul per group.
#  - AV is v-stationary [v|1] (65 cols) -> out^T [65, t] with the
#    softmax denominator in row 64; division happens on host.
#  - 3-iteration software pipeline: scores lag 1, AV lag 3 behind the
#    projections, so exp/mask latency never stalls the PE.
import os

import numpy as np
import ml_dtypes

B, T, C, H = 512, 142, 512, 64
NCORES = 8
NB = B // NCORES            # 64 batches per core
NT = NB * T                 # 9088 tokens per core
GB = 3                      # batches per full group
NG = 22                     # 21 full groups + 1 single-batch group
NFULL = 21
SCALE = float(C) ** -0.5
TW = 65                     # out rows: H + denominator row
OSPLIT = 284                # o-copy column split between ACT and DVE

# wcst column layout (all bf16)
WQK0 = 0                    # 4 chunks of [Wq_c | Wk_c] at 128*c
WV0 = 512                   # 4 chunks of Wv_c at 512 + 64*c
MSKF0 = 768                 # full-group mask [128, 426]
MSKT0 = 1194                # tail-group mask [128, 142]
WCST_COLS = 1336

_CACHE = {}


def _group_info(g):
    if g < NFULL:
        return GB, g * (GB * T)
    return 1, NFULL * (GB * T)


def _pairs():
    # pair p covers groups (2p, 2p+1); last pair = (20, 21)
    return [(p, 2 * p, min(2, NG - 2 * p)) for p in range((NG + 1) // 2)]


def _build_nc():
    import concourse.bacc as bacc
    import concourse.mybir as mybir
    from concourse.tile import TileContext

    fp32 = mybir.dt.float32
    fp16 = mybir.dt.float16
    bf16 = mybir.dt.bfloat16
    Exp = mybir.ActivationFunctionType.Exp

    nc = bacc.Bacc(
        "TRN2",
        target_bir_lowering=False,
        debug=False,
        enable_asserts=False,
        num_devices=NCORES,
    )

    xt = nc.dram_tensor("xt", [4, 128, NT], bf16, kind="ExternalInput").ap()
    # weights and masks staged separately so the critical first transfer
    # (wqkv, SP queue) is small; masks follow on the ACT queue.
    wqkv = nc.dram_tensor("wqkv", [128, 768], bf16, kind="ExternalInput").ap()
    msk = nc.dram_tensor("msk", [128, WCST_COLS - 768], bf16,
                         kind="ExternalInput").ap()
    om = nc.dram_tensor("om", [NG, TW, 468], fp16,
                        kind="ExternalOutput").ap()

    with TileContext(nc) as tc:
        with (
            tc.tile_pool(name="const", bufs=1) as cpool,
            tc.tile_pool(name="xtp", bufs=6) as xpool,
            tc.tile_pool(name="work", bufs=3) as wpool,
            tc.tile_pool(name="psum", bufs=1, space="PSUM") as ppool,
        ):
            wcst_sb = cpool.tile([128, WCST_COLS], bf16)
            # SWDGE path: its descriptor generation (Pool) runs in parallel
            # with the HWDGE generation of the first x transfers; wqk first
            # (the only blocker of the first matmul), wv right behind
            nc.gpsimd.dma_start(out=wcst_sb[:, 0:512], in_=wqkv[:, 0:512])
            nc.gpsimd.dma_start(out=wcst_sb[:, 512:768], in_=wqkv[:, 512:768])

            def wqk_c(c):
                return wcst_sb[:, WQK0 + c * 128:WQK0 + (c + 1) * 128]

            def wv_c(c):
                return wcst_sb[:, WV0 + c * 64:WV0 + (c + 1) * 64]

            xt_tiles = {}
            qsb, ksb, psc_t, exp_t, expm_t, vex_t = {}, {}, {}, {}, {}, {}
            pout_t = {}

            def issue_x_dma(g):
                nb, t0 = _group_info(g)
                gt = nb * T
                xt_t = xpool.tile([128, 4 * GB * T], bf16, tag="xt")
                xt_tiles[g] = xt_t
                if g == 0:
                    # split in halves for the first group so PE starts early
                    # (each extra DMA costs 625ns of serialized HWDGE gen)
                    for h in range(2):
                        nc.sync.dma_start(
                            out=xt_t[:, 2 * h * gt:2 * (h + 1) * gt].rearrange(
                                "p (c t) -> p c t", c=2),
                            in_=xt[2 * h:2 * (h + 1), :, 0:gt].rearrange(
                                "c p t -> p c t"))
                else:
                    nc.sync.dma_start(
                        out=xt_t[:, 0:4 * gt].rearrange(
                            "p (c t) -> p c t", c=4),
                        in_=xt[:, :, t0:t0 + gt].rearrange("c p t -> p c t"))

            def xslice(g, c, lo, hi):
                gt = _group_info(g)[0] * T
                base = c * gt
                return xt_tiles[g][:, base + lo:base + hi]

            def emit_qk(g):
                nb, _ = _group_info(g)
                gt = nb * T
                pqk = ppool.tile([128, GB * T], fp32, tag="pqk", bufs=2)
                for c in range(4):
                    nc.tensor.matmul(
                        pqk[:, 0:gt], lhsT=wqk_c(c), rhs=xslice(g, c, 0, gt),
                        start=(c == 0), stop=(c == 3))
                # one full-width PSUM->SBUF copy; rows 0:64 are qT, usable
                # directly.  kT (rows 64:128) is re-based to partition 0 by
                # a cheap SBUF->SBUF DMA so the scores operands share a base.
                qk_sb = wpool.tile([128, GB * T], bf16, tag="qk")
                k_sb = wpool.tile([64, GB * T], bf16, tag="k2")
                nc.scalar.copy(qk_sb[:, 0:gt], pqk[:, 0:gt])
                if 2 <= g < NG - 2:
                    # steady state: re-base kT via SBUF->SBUF DMA (off the
                    # compute engines; latency hidden by the 2-deep lag)
                    nc.sync.dma_start(
                        out=k_sb[0:64, 0:gt], in_=qk_sb[64:128, 0:gt])
                elif g < 2:
                    # warm-up: ACT is idle and the DMA round trip (+900ns
                    # sem) would sit on the critical path
                    nc.scalar.copy(k_sb[0:64, 0:gt], pqk[64:128, 0:gt])
                else:
                    # drain: keep the re-base on ACT, off the DVE o-chain
                    nc.scalar.copy(k_sb[0:64, 0:gt], pqk[64:128, 0:gt])
                qsb[g], ksb[g] = qk_sb, k_sb

            def emit_v(g):
                nb, _ = _group_info(g)
                pvt = ppool.tile([128, 256], fp32, tag="pvt", bufs=3)
                for j in range(nb):
                    for c in range(4):
                        nc.tensor.matmul(
                            pvt[:, j * 64:(j + 1) * 64],
                            lhsT=xslice(g, c, j * 128, (j + 1) * 128),
                            rhs=wv_c(c), start=(c == 0), stop=(c == 3))
                for c in range(4):
                    nc.tensor.matmul(
                        pvt[0:nb * 14, nb * 64:nb * 64 + 64],
                        lhsT=xslice(g, c, nb * 128, nb * 128 + nb * 14),
                        rhs=wv_c(c), start=(c == 0), stop=(c == 3))
                # one copy moves main v tiles AND the packed tail tile
                # (rows 42:126 of the tail block are dead but harmless)
                vex = wpool.tile([128, 4 * TW], bf16, tag="vex", bufs=5)
                nbl = nb + 1
                nc.vector.tensor_copy(
                    vex.rearrange("p (b h) -> p b h", h=TW)[:, 0:nbl, 0:64],
                    pvt[:, 0:nbl * 64].rearrange("p (b h) -> p b h", h=64),
                )
                if g < 5:  # ones columns: written once per pool buffer
                    nc.vector.memset(
                        vex.rearrange("p (b h) -> p b h", h=TW)[:, :, 64:65],
                        1.0)
                vex_t[g] = vex

            def emit_scores(g):
                # psc columns: [main nb*128 | sctt nb*14 | m2 nb*14]
                nb, _ = _group_info(g)
                MAIN = nb * 128
                S0, M0 = MAIN, MAIN + nb * 14
                psc = ppool.tile([128, 468], fp32, tag="psc", bufs=2)
                psc_t[g] = psc
                if g < 2:
                    # NaN-proof the never-written sctt rows once per buffer
                    # (engine partition windows: base 32 allows <=32 rows,
                    # base 64 allows <=64; rows 32:42 are rewritten by the
                    # tail-score matmuls right after)
                    nc.vector.memset(psc[32:64, S0:M0], 0.0)
                    nc.vector.memset(psc[64:128, S0:M0], 0.0)
                q_sb, k_sb = qsb.pop(g), ksb.pop(g)
                for j in range(nb):
                    nc.tensor.matmul(
                        psc[:, j * 128:(j + 1) * 128],
                        lhsT=k_sb[0:64, j * 128:(j + 1) * 128],
                        rhs=q_sb[0:64, j * 128:(j + 1) * 128],
                        start=True, stop=True)
                    nc.tensor.matmul(
                        psc[:, M0 + j * 14:M0 + (j + 1) * 14],
                        lhsT=k_sb[0:64, j * 128:(j + 1) * 128],
                        rhs=q_sb[0:64, MAIN + j * 14:MAIN + (j + 1) * 14],
                        start=True, stop=True)
                    nc.tensor.matmul(
                        psc[0:nb * 14, S0 + j * 14:S0 + (j + 1) * 14],
                        lhsT=k_sb[0:64, MAIN:MAIN + nb * 14],
                        rhs=q_sb[0:64, MAIN + j * 14:MAIN + (j + 1) * 14],
                        start=True, stop=True)

            def emit_exp(g):
                nb, _ = _group_info(g)
                expc = nb * 156
                mw = nb * 142        # masked width: main + sctt
                psc = psc_t.pop(g)
                exp_sb = wpool.tile([128, 468], bf16, tag="exp", bufs=3)
                expm = wpool.tile([128, 426], bf16, tag="expm", bufs=3)
                nc.scalar.activation(
                    exp_sb[:, 0:expc], psc[:, 0:expc], Exp, scale=SCALE)
                mo = MSKF0 if nb == GB else MSKT0
                nc.vector.tensor_mul(
                    expm[:, 0:mw], exp_sb[:, 0:mw], wcst_sb[:, mo:mo + mw])
                exp_t[g], expm_t[g] = exp_sb, expm

            def emit_av(g):
                nb, _ = _group_info(g)
                MAIN = nb * 128
                S0, M0 = MAIN, MAIN + nb * 14
                exp_sb, expm = exp_t.pop(g), expm_t.pop(g)
                vex = vex_t.pop(g)
                if g == NG - 1:
                    # psc banks are idle during the drain; a fresh tile
                    # avoids the pout WAR against o-copy(g-1)
                    pout = ppool.tile([128, 468], fp32, tag="psc", bufs=2)
                else:
                    pout = ppool.tile([TW, 468], fp32, tag="pout", bufs=1)
                pout_t[g] = pout
                for j in range(nb):
                    nc.tensor.matmul(
                        pout[0:TW, j * 128:(j + 1) * 128],
                        lhsT=vex[:, j * TW:(j + 1) * TW],
                        rhs=expm[0:128, j * 128:(j + 1) * 128],
                        start=True, stop=True)
                for j in range(nb):
                    nc.tensor.matmul(
                        pout[0:TW, MAIN + j * 14:MAIN + (j + 1) * 14],
                        lhsT=vex[:, j * TW:(j + 1) * TW],
                        rhs=exp_sb[0:128, M0 + j * 14:M0 + (j + 1) * 14],
                        start=True, stop=True)
                # tail-key contributions land in their own columns
                # (M0:M0+nb*14); the host adds the two partial sums
                nc.tensor.matmul(
                    pout[0:TW, M0:M0 + nb * 14],
                    lhsT=vex[0:nb * 14, nb * TW:(nb + 1) * TW],
                    rhs=expm[0:nb * 14, S0:M0],
                    start=True, stop=True)

            def emit_out(g):
                nb, _ = _group_info(g)
                ow = nb * 156
                pout = pout_t.pop(g)
                o_sb = wpool.tile([TW, 468], fp16, tag="o", bufs=3)
                nc.vector.tensor_copy(o_sb[0:TW, 0:ow], pout[0:TW, 0:ow])
                eng = nc.sync if g >= NG - 5 else nc.gpsimd
                eng.dma_start(out=om[g, :, 0:ow], in_=o_sb[0:TW, 0:ow])

            # software pipeline: iteration i runs
            #   qk(i), AV(i-4), out(i-4), scores(i-2), exp/mask(i-2), v(i)
            # with compressed lags over the last groups (drain phase: the
            # PE is idle there, so latency-hiding lags only stretch the
            # tail)
            sc_at = {g: g + 2 for g in range(NG)}
            av_at = {g: g + 4 for g in range(NG)}
            last_i = max(av_at.values())
            for g in range(3):
                issue_x_dma(g)
            for i in range(last_i + 1):
                if i + 3 < NG:
                    issue_x_dma(i + 3)
                if i == 1:
                    # masks are first read by mask(0) at iteration 2;
                    # deferring this transfer keeps the early DMA queue
                    # clear for x chunks and the first k re-base DMAs
                    nc.scalar.dma_start(
                        out=wcst_sb[:, 768:WCST_COLS], in_=msk)
                if i < NG:
                    emit_qk(i)
                for g in range(NG):
                    if av_at[g] == i:
                        emit_av(g)
                        emit_out(g)
                for g in range(NG):
                    if sc_at[g] == i:
                        emit_scores(g)
                        emit_exp(g)
                if i < NG:
                    emit_v(i)

    nc.compile()
    return nc


def _prep_shared(Wq, Wk, Wv):
    bf16 = ml_dtypes.bfloat16
    wqkv = np.zeros((128, 768), np.float32)
    for c in range(4):
        wqkv[:, WQK0 + c * 128:WQK0 + c * 128 + 64] = Wq[c * 128:(c + 1) * 128]
        wqkv[:, WQK0 + c * 128 + 64:WQK0 + (c + 1) * 128] = \
            Wk[c * 128:(c + 1) * 128]
        wqkv[:, WV0 + c * 64:WV0 + (c + 1) * 64] = Wv[c * 128:(c + 1) * 128]

    s = np.arange(128)[:, None]
    t = np.arange(128)[None, :]
    tri128 = (s <= t).astype(np.float32)          # valid (unmasked) = 1
    s14 = np.arange(14)[:, None]
    t14 = np.arange(14)[None, :]
    tri14 = (s14 <= t14).astype(np.float32)

    msk = np.zeros((128, WCST_COLS - 768), np.float32)
    mf = MSKF0 - 768
    for j in range(3):
        msk[:, mf + j * 128:mf + (j + 1) * 128] = tri128
    for j in range(3):
        msk[14 * j:14 * (j + 1), mf + 384 + 14 * j:mf + 384 + 14 * (j + 1)] \
            = tri14
    mt = MSKT0 - 768
    msk[:, mt:mt + 128] = tri128
    msk[0:14, mt + 128:mt + 142] = tri14
    return dict(wqkv=wqkv.astype(bf16), msk=msk.astype(bf16))


def _perm():
    # reordered token index -> natural (b*T + t) index, per core
    idx = []
    for g in range(NFULL):
        for j in range(GB):
            b = GB * g + j
            idx.append(np.arange(b * T, b * T + 128))
        for j in range(GB):
            b = GB * g + j
            idx.append(np.arange(b * T + 128, (b + 1) * T))
    b = NB - 1
    idx.append(np.arange(b * T, b * T + 128))
    idx.append(np.arange(b * T + 128, (b + 1) * T))
    return np.concatenate(idx)


_PERM = _perm()


def _prep_core_xt(x_core):
    # x_core [NB, T, C] fp32 -> [4, 128, NT] bf16 (x^T, reordered tokens)
    xall = x_core.reshape(NT, C)[_PERM]
    xtr = np.ascontiguousarray(xall.T).reshape(4, 128, NT)
    return xtr.astype(ml_dtypes.bfloat16)


def _assemble_core(om_np):
    # om [NG, 65, 468] f16 -> [NB, T, H] normalized fp32
    # tail-query columns hold two partial sums (main keys at 384:426,
    # tail keys at 426:468) that are added here
    om_np = om_np.astype(np.float32)
    full = om_np[:NFULL]                                 # [21, 65, 468]
    mains = full[:, :, 0:384].reshape(NFULL, TW, 3, 128)
    mains = mains.transpose(0, 2, 3, 1).reshape(63, 128, TW)
    tails = (full[:, :, 384:426] + full[:, :, 426:468]).reshape(
        NFULL, TW, 3, 14)
    tails = tails.transpose(0, 2, 3, 1).reshape(63, 14, TW)
    toks = np.concatenate([mains, tails], axis=1)        # [63, 142, 65]
    lt = om_np[NFULL, :, 128:142] + om_np[NFULL, :, 142:156]
    last = np.concatenate(
        [om_np[NFULL, :, 0:128].T, lt.T], axis=0)[None]  # [1, 142, 65]
    allb = np.concatenate([toks, last], axis=0)          # [64, 142, 65]
    return allb[:, :, 0:H] / allb[:, :, H:H + 1]


def kernel(**inputs):
    x = np.asarray(inputs["x"], dtype=np.float32)
    Wq = np.asarray(inputs["Wq"], dtype=np.float32)
    Wk = np.asarray(inputs["Wk"], dtype=np.float32)
    Wv = np.asarray(inputs["Wv"], dtype=np.float32)

    from concourse.bass_utils import run_bass_kernel_spmd

    if "nc" not in _CACHE:
        _CACHE["nc"] = _build_nc()
    nc = _CACHE["nc"]

    shared = _prep_shared(Wq, Wk, Wv)
    in_maps = []
    for core in range(NCORES):
        m = dict(shared)
        m["xt"] = _prep_core_xt(x[core * NB:(core + 1) * NB])
        in_maps.append(m)

    trace = bool(int(os.environ.get("TRN_KERNEL_TRACE", "0")))
    res = run_bass_kernel_spmd(
        nc, in_maps, core_ids=list(range(NCORES)), trace=trace,
    )
    _CACHE["last_result"] = res

    outs = []
    for core in range(NCORES):
        r = res.results[core]
        outs.append(_assemble_core(np.asarray(r["om"])))
    return np.concatenate(outs, axis=0).astype(np.float32)


# revision 62
# speedup vs baseline: 1.4092x; 1.0059x over previous
# Trainium2 Bass kernel for single-head causal attention
#   q = x@Wq, k = x@Wk, v = x@Wv   (x [B,T,C], W* [C,H])
#   out = softmax(mask(q k^T / sqrt(C))) @ v
# B=512, T=142, C=512, H=64.  Data-parallel over B across 8 NeuronCores.
#
# Per-core layout (64 batches, 21 groups of 3 + 1 tail group):
#  - host reorders tokens per group: [b0 0:128 | b1 0:128 | b2 0:128 |
#    b0 128:142 | b1 128:142 | b2 128:142] so every PE operand is a
#    contiguous column range.
#  - [Wq|Wk] packed stationary -> one PSUM accumulation; qT rows 0:64,
#    kT rows 64:128; a partition-shifted copy re-bases k to rows 0:64.
#  - v computed x-stationary directly in natural [token, h] layout;
#    the 3 batch tails share one 42-column stationary tile; all v tiles
#    leave PSUM in a single engine copy.
#  - causal mask applied as a 0/1 bf16 multiply on DVE after exp
#    (no mask matmuls on the PE); main-keys-x-tail-queries columns are
#    fully valid and skip the mask entirely.
#  - tail scores use all 3 batches' key-tails as one stationary; the
#    cross-batch terms are zeroed by the mask multiply, enabling ONE
#    fused tail-AV matmul per group.
#  - AV is v-stationary [v|1] (65 cols) -> out^T [65, t] with the
#    softmax denominator in row 64; division happens on host.
#  - 3-iteration software pipeline: scores lag 1, AV lag 3 behind the
#    projections, so exp/mask latency never stalls the PE.
import os

import numpy as np
import ml_dtypes

B, T, C, H = 512, 142, 512, 64
NCORES = 8
NB = B // NCORES            # 64 batches per core
NT = NB * T                 # 9088 tokens per core
GB = 3                      # batches per full group
NG = 22                     # 21 full groups + 1 single-batch group
NFULL = 21
SCALE = float(C) ** -0.5
TW = 65                     # out rows: H + denominator row
OSPLIT = 284                # o-copy column split between ACT and DVE

# wcst column layout (all bf16)
WQK0 = 0                    # 4 chunks of [Wq_c | Wk_c] at 128*c
WV0 = 512                   # 4 chunks of Wv_c at 512 + 64*c
MSKF0 = 768                 # full-group mask [128, 426]
MSKT0 = 1194                # tail-group mask [128, 142]
WCST_COLS = 1336

_CACHE = {}


def _group_info(g):
    if g < NFULL:
        return GB, g * (GB * T)
    return 1, NFULL * (GB * T)


def _pairs():
    # pair p covers groups (2p, 2p+1); last pair = (20, 21)
    return [(p, 2 * p, min(2, NG - 2 * p)) for p in range((NG + 1) // 2)]


def _build_nc():
    import concourse.bacc as bacc
    import concourse.mybir as mybir
    from concourse.tile import TileContext

    fp32 = mybir.dt.float32
    fp16 = mybir.dt.float16
    bf16 = mybir.dt.bfloat16
    Exp = mybir.ActivationFunctionType.Exp

    nc = bacc.Bacc(
        "TRN2",
        target_bir_lowering=False,
        debug=False,
        enable_asserts=False,
        num_devices=NCORES,
    )

    xt = nc.dram_tensor("xt", [4, 128, NT], bf16, kind="ExternalInput").ap()
    # weights and masks staged separately so the critical first transfer
    # (wqkv, SP queue) is small; masks follow on the ACT queue.
    wqkv = nc.dram_tensor("wqkv", [128, 768], bf16, kind="ExternalInput").ap()
    msk = nc.dram_tensor("msk", [128, WCST_COLS - 768], bf16,
                         kind="ExternalInput").ap()
    om = nc.dram_tensor("om", [NG, TW, 468], fp16,
                        kind="ExternalOutput").ap()

    with TileContext(nc) as tc:
        with (
            tc.tile_pool(name="const", bufs=1) as cpool,
            tc.tile_pool(name="xtp", bufs=6) as xpool,
            tc.tile_pool(name="work", bufs=3) as wpool,
            tc.tile_pool(name="psum", bufs=1, space="PSUM") as ppool,
        ):
            wcst_sb = cpool.tile([128, WCST_COLS], bf16)
            # SWDGE path: its descriptor generation (Pool) runs in parallel
            # with the HWDGE generation of the first x transfers; wqk first
            # (the only blocker of the first matmul), wv right behind
            nc.gpsimd.dma_start(out=wcst_sb[:, 0:512], in_=wqkv[:, 0:512])
            nc.gpsimd.dma_start(out=wcst_sb[:, 512:768], in_=wqkv[:, 512:768])

            def wqk_c(c):
                return wcst_sb[:, WQK0 + c * 128:WQK0 + (c + 1) * 128]

            def wv_c(c):
                return wcst_sb[:, WV0 + c * 64:WV0 + (c + 1) * 64]

            xt_tiles = {}
            qsb, ksb, psc_t, exp_t, expm_t, vex_t = {}, {}, {}, {}, {}, {}
            pout_t = {}

            def issue_x_dma(g):
                nb, t0 = _group_info(g)
                gt = nb * T
                xt_t = xpool.tile([128, 4 * GB * T], bf16, tag="xt")
                xt_tiles[g] = xt_t
                if g == 0:
                    # split in halves for the first group so PE starts early
                    # (each extra DMA costs 625ns of serialized HWDGE gen)
                    for h in range(2):
                        nc.sync.dma_start(
                            out=xt_t[:, 2 * h * gt:2 * (h + 1) * gt].rearrange(
                                "p (c t) -> p c t", c=2),
                            in_=xt[2 * h:2 * (h + 1), :, 0:gt].rearrange(
                                "c p t -> p c t"))
                else:
                    nc.sync.dma_start(
                        out=xt_t[:, 0:4 * gt].rearrange(
                            "p (c t) -> p c t", c=4),
                        in_=xt[:, :, t0:t0 + gt].rearrange("c p t -> p c t"))

            def xslice(g, c, lo, hi):
                gt = _group_info(g)[0] * T
                base = c * gt
                return xt_tiles[g][:, base + lo:base + hi]

            def emit_qk(g):
                nb, _ = _group_info(g)
                gt = nb * T
                pqk = ppool.tile([128, GB * T], fp32, tag="pqk", bufs=2)
                for c in range(4):
                    nc.tensor.matmul(
                        pqk[:, 0:gt], lhsT=wqk_c(c), rhs=xslice(g, c, 0, gt),
                        start=(c == 0), stop=(c == 3))
                # one full-width PSUM->SBUF copy; rows 0:64 are qT, usable
                # directly.  kT (rows 64:128) is re-based to partition 0 by
                # a cheap SBUF->SBUF DMA so the scores operands share a base.
                qk_sb = wpool.tile([128, GB * T], bf16, tag="qk", bufs=4)
                k_sb = wpool.tile([64, GB * T], bf16, tag="k2", bufs=4)
                nc.scalar.copy(qk_sb[:, 0:gt], pqk[:, 0:gt])
                if 2 <= g < NG - 2:
                    # steady state: re-base kT via SBUF->SBUF DMA (off the
                    # compute engines; latency hidden by the 2-deep lag)
                    nc.sync.dma_start(
                        out=k_sb[0:64, 0:gt], in_=qk_sb[64:128, 0:gt])
                elif g < 2:
                    # warm-up: ACT is idle and the DMA round trip (+900ns
                    # sem) would sit on the critical path
                    nc.scalar.copy(k_sb[0:64, 0:gt], pqk[64:128, 0:gt])
                else:
                    # drain: keep the re-base on ACT, off the DVE o-chain
                    nc.scalar.copy(k_sb[0:64, 0:gt], pqk[64:128, 0:gt])
                qsb[g], ksb[g] = qk_sb, k_sb

            def emit_v(g):
                nb, _ = _group_info(g)
                pvt = ppool.tile([128, 256], fp32, tag="pvt", bufs=3)
                for j in range(nb):
                    for c in range(4):
                        nc.tensor.matmul(
                            pvt[:, j * 64:(j + 1) * 64],
                            lhsT=xslice(g, c, j * 128, (j + 1) * 128),
                            rhs=wv_c(c), start=(c == 0), stop=(c == 3))
                for c in range(4):
                    nc.tensor.matmul(
                        pvt[0:nb * 14, nb * 64:nb * 64 + 64],
                        lhsT=xslice(g, c, nb * 128, nb * 128 + nb * 14),
                        rhs=wv_c(c), start=(c == 0), stop=(c == 3))
                # one copy moves main v tiles AND the packed tail tile
                # (rows 42:126 of the tail block are dead but harmless)
                vex = wpool.tile([128, 4 * TW], bf16, tag="vex", bufs=6)
                nbl = nb + 1
                nc.vector.tensor_copy(
                    vex.rearrange("p (b h) -> p b h", h=TW)[:, 0:nbl, 0:64],
                    pvt[:, 0:nbl * 64].rearrange("p (b h) -> p b h", h=64),
                )
                if g < 6:  # ones columns: written once per pool buffer
                    nc.vector.memset(
                        vex.rearrange("p (b h) -> p b h", h=TW)[:, :, 64:65],
                        1.0)
                vex_t[g] = vex

            def emit_scores(g):
                # psc columns: [main nb*128 | sctt nb*14 | m2 nb*14]
                nb, _ = _group_info(g)
                MAIN = nb * 128
                S0, M0 = MAIN, MAIN + nb * 14
                psc = ppool.tile([128, 468], fp32, tag="psc", bufs=2)
                psc_t[g] = psc
                if g < 2:
                    # NaN-proof the never-written sctt rows once per buffer
                    # (engine partition windows: base 32 allows <=32 rows,
                    # base 64 allows <=64; rows 32:42 are rewritten by the
                    # tail-score matmuls right after)
                    nc.vector.memset(psc[32:64, S0:M0], 0.0)
                    nc.vector.memset(psc[64:128, S0:M0], 0.0)
                q_sb, k_sb = qsb.pop(g), ksb.pop(g)
                for j in range(nb):
                    nc.tensor.matmul(
                        psc[:, j * 128:(j + 1) * 128],
                        lhsT=k_sb[0:64, j * 128:(j + 1) * 128],
                        rhs=q_sb[0:64, j * 128:(j + 1) * 128],
                        start=True, stop=True)
                    nc.tensor.matmul(
                        psc[:, M0 + j * 14:M0 + (j + 1) * 14],
                        lhsT=k_sb[0:64, j * 128:(j + 1) * 128],
                        rhs=q_sb[0:64, MAIN + j * 14:MAIN + (j + 1) * 14],
                        start=True, stop=True)
                    nc.tensor.matmul(
                        psc[0:nb * 14, S0 + j * 14:S0 + (j + 1) * 14],
                        lhsT=k_sb[0:64, MAIN:MAIN + nb * 14],
                        rhs=q_sb[0:64, MAIN + j * 14:MAIN + (j + 1) * 14],
                        start=True, stop=True)

            def emit_exp(g):
                nb, _ = _group_info(g)
                expc = nb * 156
                mw = nb * 142        # masked width: main + sctt
                psc = psc_t.pop(g)
                exp_sb = wpool.tile([128, 468], bf16, tag="exp", bufs=4)
                expm = wpool.tile([128, 426], bf16, tag="expm", bufs=4)
                nc.scalar.activation(
                    exp_sb[:, 0:expc], psc[:, 0:expc], Exp, scale=SCALE)
                mo = MSKF0 if nb == GB else MSKT0
                nc.vector.tensor_mul(
                    expm[:, 0:mw], exp_sb[:, 0:mw], wcst_sb[:, mo:mo + mw])
                exp_t[g], expm_t[g] = exp_sb, expm

            def emit_av(g):
                nb, _ = _group_info(g)
                MAIN = nb * 128
                S0, M0 = MAIN, MAIN + nb * 14
                exp_sb, expm = exp_t.pop(g), expm_t.pop(g)
                vex = vex_t.pop(g)
                if g == NG - 1:
                    # psc banks are idle during the drain; a fresh tile
                    # avoids the pout WAR against o-copy(g-1)
                    pout = ppool.tile([128, 468], fp32, tag="psc", bufs=2)
                else:
                    pout = ppool.tile([TW, 468], fp32, tag="pout", bufs=1)
                pout_t[g] = pout
                for j in range(nb):
                    nc.tensor.matmul(
                        pout[0:TW, j * 128:(j + 1) * 128],
                        lhsT=vex[:, j * TW:(j + 1) * TW],
                        rhs=expm[0:128, j * 128:(j + 1) * 128],
                        start=True, stop=True)
                for j in range(nb):
                    nc.tensor.matmul(
                        pout[0:TW, MAIN + j * 14:MAIN + (j + 1) * 14],
                        lhsT=vex[:, j * TW:(j + 1) * TW],
                        rhs=exp_sb[0:128, M0 + j * 14:M0 + (j + 1) * 14],
                        start=True, stop=True)
                # tail-key contributions land in their own columns
                # (M0:M0+nb*14); the host adds the two partial sums
                nc.tensor.matmul(
                    pout[0:TW, M0:M0 + nb * 14],
                    lhsT=vex[0:nb * 14, nb * TW:(nb + 1) * TW],
                    rhs=expm[0:nb * 14, S0:M0],
                    start=True, stop=True)

            def emit_out(g):
                nb, _ = _group_info(g)
                ow = nb * 156
                pout = pout_t.pop(g)
                o_sb = wpool.tile([TW, 468], fp16, tag="o", bufs=4)
                nc.vector.tensor_copy(o_sb[0:TW, 0:ow], pout[0:TW, 0:ow])
                eng = nc.sync if g >= NG - 5 else nc.gpsimd
                eng.dma_start(out=om[g, :, 0:ow], in_=o_sb[0:TW, 0:ow])

            # software pipeline: iteration i runs
            #   qk(i), AV(i-4), out(i-4), scores(i-2), exp/mask(i-2), v(i)
            # with compressed lags over the last groups (drain phase: the
            # PE is idle there, so latency-hiding lags only stretch the
            # tail)
            sc_at = {g: g + 2 for g in range(NG)}
            av_at = {g: g + 4 for g in range(NG)}
            last_i = max(av_at.values())
            for g in range(3):
                issue_x_dma(g)
            for i in range(last_i + 1):
                if i + 3 < NG:
                    issue_x_dma(i + 3)
                if i == 1:
                    # masks are first read by mask(0) at iteration 2;
                    # deferring this transfer keeps the early DMA queue
                    # clear for x chunks and the first k re-base DMAs
                    nc.scalar.dma_start(
                        out=wcst_sb[:, 768:WCST_COLS], in_=msk)
                if i < NG:
                    emit_qk(i)
                for g in range(NG):
                    if av_at[g] == i:
                        emit_av(g)
                        emit_out(g)
                for g in range(NG):
                    if sc_at[g] == i:
                        emit_scores(g)
                        emit_exp(g)
                if i < NG:
                    emit_v(i)

    nc.compile()
    return nc


def _prep_shared(Wq, Wk, Wv):
    bf16 = ml_dtypes.bfloat16
    wqkv = np.zeros((128, 768), np.float32)
    for c in range(4):
        wqkv[:, WQK0 + c * 128:WQK0 + c * 128 + 64] = Wq[c * 128:(c + 1) * 128]
        wqkv[:, WQK0 + c * 128 + 64:WQK0 + (c + 1) * 128] = \
            Wk[c * 128:(c + 1) * 128]
        wqkv[:, WV0 + c * 64:WV0 + (c + 1) * 64] = Wv[c * 128:(c + 1) * 128]

    s = np.arange(128)[:, None]
    t = np.arange(128)[None, :]
    tri128 = (s <= t).astype(np.float32)          # valid (unmasked) = 1
    s14 = np.arange(14)[:, None]
    t14 = np.arange(14)[None, :]
    tri14 = (s14 <= t14).astype(np.float32)

    msk = np.zeros((128, WCST_COLS - 768), np.float32)
    mf = MSKF0 - 768
    for j in range(3):
        msk[:, mf + j * 128:mf + (j + 1) * 128] = tri128
    for j in range(3):
        msk[14 * j:14 * (j + 1), mf + 384 + 14 * j:mf + 384 + 14 * (j + 1)] \
            = tri14
    mt = MSKT0 - 768
    msk[:, mt:mt + 128] = tri128
    msk[0:14, mt + 128:mt + 142] = tri14
    return dict(wqkv=wqkv.astype(bf16), msk=msk.astype(bf16))


def _perm():
    # reordered token index -> natural (b*T + t) index, per core
    idx = []
    for g in range(NFULL):
        for j in range(GB):
            b = GB * g + j
            idx.append(np.arange(b * T, b * T + 128))
        for j in range(GB):
            b = GB * g + j
            idx.append(np.arange(b * T + 128, (b + 1) * T))
    b = NB - 1
    idx.append(np.arange(b * T, b * T + 128))
    idx.append(np.arange(b * T + 128, (b + 1) * T))
    return np.concatenate(idx)


_PERM = _perm()


def _prep_core_xt(x_core):
    # x_core [NB, T, C] fp32 -> [4, 128, NT] bf16 (x^T, reordered tokens)
    xall = x_core.reshape(NT, C)[_PERM]
    xtr = np.ascontiguousarray(xall.T).reshape(4, 128, NT)
    return xtr.astype(ml_dtypes.bfloat16)


def _assemble_core(om_np):
    # om [NG, 65, 468] f16 -> [NB, T, H] normalized fp32
    # tail-query columns hold two partial sums (main keys at 384:426,
    # tail keys at 426:468) that are added here
    om_np = om_np.astype(np.float32)
    full = om_np[:NFULL]                                 # [21, 65, 468]
    mains = full[:, :, 0:384].reshape(NFULL, TW, 3, 128)
    mains = mains.transpose(0, 2, 3, 1).reshape(63, 128, TW)
    tails = (full[:, :, 384:426] + full[:, :, 426:468]).reshape(
        NFULL, TW, 3, 14)
    tails = tails.transpose(0, 2, 3, 1).reshape(63, 14, TW)
    toks = np.concatenate([mains, tails], axis=1)        # [63, 142, 65]
    lt = om_np[NFULL, :, 128:142] + om_np[NFULL, :, 142:156]
    last = np.concatenate(
        [om_np[NFULL, :, 0:128].T, lt.T], axis=0)[None]  # [1, 142, 65]
    allb = np.concatenate([toks, last], axis=0)          # [64, 142, 65]
    return allb[:, :, 0:H] / allb[:, :, H:H + 1]


def kernel(**inputs):
    x = np.asarray(inputs["x"], dtype=np.float32)
    Wq = np.asarray(inputs["Wq"], dtype=np.float32)
    Wk = np.asarray(inputs["Wk"], dtype=np.float32)
    Wv = np.asarray(inputs["Wv"], dtype=np.float32)

    from concourse.bass_utils import run_bass_kernel_spmd

    if "nc" not in _CACHE:
        _CACHE["nc"] = _build_nc()
    nc = _CACHE["nc"]

    shared = _prep_shared(Wq, Wk, Wv)
    in_maps = []
    for core in range(NCORES):
        m = dict(shared)
        m["xt"] = _prep_core_xt(x[core * NB:(core + 1) * NB])
        in_maps.append(m)

    trace = bool(int(os.environ.get("TRN_KERNEL_TRACE", "0")))
    res = run_bass_kernel_spmd(
        nc, in_maps, core_ids=list(range(NCORES)), trace=trace,
    )
    _CACHE["last_result"] = res

    outs = []
    for core in range(NCORES):
        r = res.results[core]
        outs.append(_assemble_core(np.asarray(r["om"])))
    return np.concatenate(outs, axis=0).astype(np.float32)


# revision 76
# speedup vs baseline: 1.4392x; 1.0213x over previous
# Trainium2 Bass kernel for single-head causal attention
#   q = x@Wq, k = x@Wk, v = x@Wv   (x [B,T,C], W* [C,H])
#   out = softmax(mask(q k^T / sqrt(C))) @ v
# B=512, T=142, C=512, H=64.  Data-parallel over B across 8 NeuronCores.
#
# Per-core layout (64 batches, 21 groups of 3 + 1 tail group):
#  - host reorders tokens per group: [b0 0:128 | b1 0:128 | b2 0:128 |
#    b0 128:142 | b1 128:142 | b2 128:142] so every PE operand is a
#    contiguous column range.
#  - [Wq|Wk] packed stationary -> one PSUM accumulation; qT rows 0:64,
#    kT rows 64:128; one full-width PSUM->SBUF copy yields q in place,
#    and a partition-shifted engine copy (alternating ACT/DVE by group
#    parity for balance) re-bases kT to partition 0 so the scores
#    operands share a base, as the compiler requires.
#  - v computed x-stationary directly in natural [token, h] layout;
#    the 3 batch tails share one 42-column stationary tile; all v tiles
#    leave PSUM in a single engine copy.
#  - causal mask applied as a 0/1 bf16 multiply on DVE after exp
#    (no mask matmuls on the PE); main-keys-x-tail-queries columns are
#    fully valid and skip the mask entirely.
#  - tail scores use all 3 batches' key-tails as one stationary; the
#    cross-batch terms are zeroed by the mask multiply, enabling ONE
#    fused tail-AV matmul per group, written to its own PSUM columns
#    (the host adds the main-key and tail-key partial sums).
#  - AV is v-stationary [v|1] (65 cols) -> out^T [65, t] with the
#    softmax denominator in row 64; division happens on host.
#  - software pipeline: scores lag 2 and AV lag 4 behind the
#    projections, so copy/exp/mask/DMA latency never stalls the PE.
import os

import numpy as np
import ml_dtypes

B, T, C, H = 512, 142, 512, 64
NCORES = 8
NB = B // NCORES            # 64 batches per core
NT = NB * T                 # 9088 tokens per core
GB = 3                      # batches per full group
NG = 22                     # 21 full groups + 1 single-batch group
NFULL = 21
SCALE = float(C) ** -0.5
TW = 65                     # out rows: H + denominator row
# wcst column layout (all bf16)
WQK0 = 0                    # 4 chunks of [Wq_c | Wk_c] at 128*c
WV0 = 512                   # 4 chunks of Wv_c at 512 + 64*c
MSKF0 = 768                 # full-group mask [128, 426]
MSKT0 = 1194                # tail-group mask [128, 142]
WCST_COLS = 1336

_CACHE = {}


def _group_info(g):
    if g < NFULL:
        return GB, g * (GB * T)
    return 1, NFULL * (GB * T)


def _build_nc():
    import concourse.bacc as bacc
    import concourse.mybir as mybir
    from concourse.tile import TileContext

    fp32 = mybir.dt.float32
    fp16 = mybir.dt.float16
    bf16 = mybir.dt.bfloat16
    Exp = mybir.ActivationFunctionType.Exp

    nc = bacc.Bacc(
        "TRN2",
        target_bir_lowering=False,
        debug=False,
        enable_asserts=False,
        num_devices=NCORES,
    )

    xt = nc.dram_tensor("xt", [4, 128, NT], bf16, kind="ExternalInput").ap()
    # weights and masks staged separately so the critical first transfer
    # (wqkv, SP queue) is small; masks follow on the ACT queue.
    wqkv = nc.dram_tensor("wqkv", [128, 768], bf16, kind="ExternalInput").ap()
    msk = nc.dram_tensor("msk", [128, WCST_COLS - 768], bf16,
                         kind="ExternalInput").ap()
    om = nc.dram_tensor("om", [NG, TW, 468], fp16,
                        kind="ExternalOutput").ap()

    with TileContext(nc) as tc:
        with (
            tc.tile_pool(name="const", bufs=1) as cpool,
            tc.tile_pool(name="xtp", bufs=6) as xpool,
            tc.tile_pool(name="work", bufs=3) as wpool,
            tc.tile_pool(name="psum", bufs=1, space="PSUM") as ppool,
        ):
            wcst_sb = cpool.tile([128, WCST_COLS], bf16)
            # SWDGE path: its descriptor generation (Pool) runs in parallel
            # with the HWDGE generation of the first x transfers; wqk first
            # (the only blocker of the first matmul), wv right behind
            nc.gpsimd.dma_start(out=wcst_sb[:, 0:512], in_=wqkv[:, 0:512])
            nc.gpsimd.dma_start(out=wcst_sb[:, 512:768], in_=wqkv[:, 512:768])

            def wqk_c(c):
                return wcst_sb[:, WQK0 + c * 128:WQK0 + (c + 1) * 128]

            def wv_c(c):
                return wcst_sb[:, WV0 + c * 64:WV0 + (c + 1) * 64]

            xt_tiles = {}
            qsb, ksb, psc_t, exp_t, expm_t, vex_t = {}, {}, {}, {}, {}, {}
            pout_t = {}

            def issue_x_dma(g):
                nb, t0 = _group_info(g)
                gt = nb * T
                xt_t = xpool.tile([128, 4 * GB * T], bf16, tag="xt")
                xt_tiles[g] = xt_t
                if g == 0:
                    # split in halves for the first group so PE starts early
                    # (each extra DMA costs 625ns of serialized HWDGE gen)
                    for h in range(2):
                        nc.sync.dma_start(
                            out=xt_t[:, 2 * h * gt:2 * (h + 1) * gt].rearrange(
                                "p (c t) -> p c t", c=2),
                            in_=xt[2 * h:2 * (h + 1), :, 0:gt].rearrange(
                                "c p t -> p c t"))
                else:
                    nc.sync.dma_start(
                        out=xt_t[:, 0:4 * gt].rearrange(
                            "p (c t) -> p c t", c=4),
                        in_=xt[:, :, t0:t0 + gt].rearrange("c p t -> p c t"))

            def xslice(g, c, lo, hi):
                gt = _group_info(g)[0] * T
                base = c * gt
                return xt_tiles[g][:, base + lo:base + hi]

            def emit_qk(g):
                nb, _ = _group_info(g)
                gt = nb * T
                pqk = ppool.tile([128, GB * T], fp32, tag="pqk", bufs=2)
                for c in range(4):
                    nc.tensor.matmul(
                        pqk[:, 0:gt], lhsT=wqk_c(c), rhs=xslice(g, c, 0, gt),
                        start=(c == 0), stop=(c == 3))
                # one full-width PSUM->SBUF copy; rows 0:64 are qT, usable
                # directly.  kT (rows 64:128) is re-based to partition 0 by
                # a partition-shifted engine copy so the scores operands
                # share a base (a hard compiler requirement).
                qk_sb = wpool.tile([128, GB * T], bf16, tag="qk", bufs=4)
                k_sb = wpool.tile([64, GB * T], bf16, tag="k2", bufs=4)
                nc.scalar.copy(qk_sb[:, 0:gt], pqk[:, 0:gt])
                if 2 <= g < NG - 2:
                    # alternate the shifted re-base copy between engines
                    eng = nc.scalar.copy if g % 2 else nc.vector.tensor_copy
                    eng(k_sb[0:64, 0:gt], pqk[64:128, 0:gt])
                elif g < 2:
                    # warm-up: ACT is idle and the DMA round trip (+900ns
                    # sem) would sit on the critical path
                    nc.scalar.copy(k_sb[0:64, 0:gt], pqk[64:128, 0:gt])
                else:
                    # drain: keep the re-base on ACT, off the DVE o-chain
                    nc.scalar.copy(k_sb[0:64, 0:gt], pqk[64:128, 0:gt])
                qsb[g], ksb[g] = qk_sb, k_sb

            def emit_v(g):
                nb, _ = _group_info(g)
                pvt = ppool.tile([128, 256], fp32, tag="pvt", bufs=3)
                for j in range(nb):
                    for c in range(4):
                        nc.tensor.matmul(
                            pvt[:, j * 64:(j + 1) * 64],
                            lhsT=xslice(g, c, j * 128, (j + 1) * 128),
                            rhs=wv_c(c), start=(c == 0), stop=(c == 3))
                for c in range(4):
                    nc.tensor.matmul(
                        pvt[0:nb * 14, nb * 64:nb * 64 + 64],
                        lhsT=xslice(g, c, nb * 128, nb * 128 + nb * 14),
                        rhs=wv_c(c), start=(c == 0), stop=(c == 3))
                # one copy moves main v tiles AND the packed tail tile
                # (rows 42:126 of the tail block are dead but harmless)
                vex = wpool.tile([128, 4 * TW], bf16, tag="vex", bufs=6)
                nbl = nb + 1
                nc.vector.tensor_copy(
                    vex.rearrange("p (b h) -> p b h", h=TW)[:, 0:nbl, 0:64],
                    pvt[:, 0:nbl * 64].rearrange("p (b h) -> p b h", h=64),
                )
                if g < 6:  # ones columns: written once per pool buffer
                    nc.vector.memset(
                        vex.rearrange("p (b h) -> p b h", h=TW)[:, :, 64:65],
                        1.0)
                vex_t[g] = vex

            def emit_scores(g):
                # psc columns: [main nb*128 | sctt nb*14 | m2 nb*14]
                nb, _ = _group_info(g)
                MAIN = nb * 128
                S0, M0 = MAIN, MAIN + nb * 14
                psc = ppool.tile([128, 468], fp32, tag="psc", bufs=2)
                psc_t[g] = psc
                if g < 2:
                    # NaN-proof the never-written sctt rows once per buffer
                    # (engine partition windows: base 32 allows <=32 rows,
                    # base 64 allows <=64; rows 32:42 are rewritten by the
                    # tail-score matmuls right after)
                    nc.vector.memset(psc[32:64, S0:M0], 0.0)
                    nc.vector.memset(psc[64:128, S0:M0], 0.0)
                q_sb, k_sb = qsb.pop(g), ksb.pop(g)
                for j in range(nb):
                    nc.tensor.matmul(
                        psc[:, j * 128:(j + 1) * 128],
                        lhsT=k_sb[0:64, j * 128:(j + 1) * 128],
                        rhs=q_sb[0:64, j * 128:(j + 1) * 128],
                        start=True, stop=True)
                    nc.tensor.matmul(
                        psc[:, M0 + j * 14:M0 + (j + 1) * 14],
                        lhsT=k_sb[0:64, j * 128:(j + 1) * 128],
                        rhs=q_sb[0:64, MAIN + j * 14:MAIN + (j + 1) * 14],
                        start=True, stop=True)
                    nc.tensor.matmul(
                        psc[0:nb * 14, S0 + j * 14:S0 + (j + 1) * 14],
                        lhsT=k_sb[0:64, MAIN:MAIN + nb * 14],
                        rhs=q_sb[0:64, MAIN + j * 14:MAIN + (j + 1) * 14],
                        start=True, stop=True)

            def emit_exp(g):
                nb, _ = _group_info(g)
                expc = nb * 156
                mw = nb * 142        # masked width: main + sctt
                psc = psc_t.pop(g)
                exp_sb = wpool.tile([128, 468], bf16, tag="exp", bufs=4)
                expm = wpool.tile([128, 426], bf16, tag="expm", bufs=4)
                nc.scalar.activation(
                    exp_sb[:, 0:expc], psc[:, 0:expc], Exp, scale=SCALE)
                mo = MSKF0 if nb == GB else MSKT0
                nc.vector.tensor_mul(
                    expm[:, 0:mw], exp_sb[:, 0:mw], wcst_sb[:, mo:mo + mw])
                exp_t[g], expm_t[g] = exp_sb, expm

            def emit_av(g):
                nb, _ = _group_info(g)
                MAIN = nb * 128
                S0, M0 = MAIN, MAIN + nb * 14
                exp_sb, expm = exp_t.pop(g), expm_t.pop(g)
                vex = vex_t.pop(g)
                if g == NG - 1:
                    # psc banks are idle during the drain; a fresh tile
                    # avoids the pout WAR against o-copy(g-1)
                    pout = ppool.tile([128, 468], fp32, tag="psc", bufs=2)
                else:
                    pout = ppool.tile([TW, 468], fp32, tag="pout", bufs=1)
                pout_t[g] = pout
                for j in range(nb):
                    nc.tensor.matmul(
                        pout[0:TW, j * 128:(j + 1) * 128],
                        lhsT=vex[:, j * TW:(j + 1) * TW],
                        rhs=expm[0:128, j * 128:(j + 1) * 128],
                        start=True, stop=True)
                for j in range(nb):
                    nc.tensor.matmul(
                        pout[0:TW, MAIN + j * 14:MAIN + (j + 1) * 14],
                        lhsT=vex[:, j * TW:(j + 1) * TW],
                        rhs=exp_sb[0:128, M0 + j * 14:M0 + (j + 1) * 14],
                        start=True, stop=True)
                # tail-key contributions land in their own columns
                # (M0:M0+nb*14); the host adds the two partial sums
                nc.tensor.matmul(
                    pout[0:TW, M0:M0 + nb * 14],
                    lhsT=vex[0:nb * 14, nb * TW:(nb + 1) * TW],
                    rhs=expm[0:nb * 14, S0:M0],
                    start=True, stop=True)

            def emit_out(g):
                nb, _ = _group_info(g)
                ow = nb * 156
                pout = pout_t.pop(g)
                o_sb = wpool.tile([TW, 468], fp16, tag="o", bufs=4)
                nc.vector.tensor_copy(o_sb[0:TW, 0:ow], pout[0:TW, 0:ow])
                eng = nc.sync if g >= NG - 5 else nc.gpsimd
                eng.dma_start(out=om[g, :, 0:ow], in_=o_sb[0:TW, 0:ow])

            # software pipeline: iteration i runs
            #   qk(i), AV(i-4), out(i-4), scores(i-2), exp/mask(i-2), v(i)
            # with compressed lags over the last groups (drain phase: the
            # PE is idle there, so latency-hiding lags only stretch the
            # tail)
            sc_at = {g: g + 2 for g in range(NG)}
            av_at = {g: g + 4 for g in range(NG)}
            last_i = max(av_at.values())
            for g in range(3):
                issue_x_dma(g)
            for i in range(last_i + 1):
                if i + 3 < NG:
                    issue_x_dma(i + 3)
                if i == 1:
                    # masks are first read by mask(0) at iteration 2;
                    # deferring this transfer keeps the early DMA queue
                    # clear for x chunks and the first k re-base DMAs
                    nc.scalar.dma_start(
                        out=wcst_sb[:, 768:WCST_COLS], in_=msk)
                if i < NG:
                    emit_qk(i)
                for g in range(NG):
                    if av_at[g] == i:
                        emit_av(g)
                        emit_out(g)
                for g in range(NG):
                    if sc_at[g] == i:
                        emit_scores(g)
                        emit_exp(g)
                if i < NG:
                    emit_v(i)

    nc.compile()
    return nc


def _prep_shared(Wq, Wk, Wv):
    bf16 = ml_dtypes.bfloat16
    wqkv = np.zeros((128, 768), np.float32)
    for c in range(4):
        wqkv[:, WQK0 + c * 128:WQK0 + c * 128 + 64] = Wq[c * 128:(c + 1) * 128]
        wqkv[:, WQK0 + c * 128 + 64:WQK0 + (c + 1) * 128] = \
            Wk[c * 128:(c + 1) * 128]
        wqkv[:, WV0 + c * 64:WV0 + (c + 1) * 64] = Wv[c * 128:(c + 1) * 128]

    s = np.arange(128)[:, None]
    t = np.arange(128)[None, :]
    tri128 = (s <= t).astype(np.float32)          # valid (unmasked) = 1
    s14 = np.arange(14)[:, None]
    t14 = np.arange(14)[None, :]
    tri14 = (s14 <= t14).astype(np.float32)

    msk = np.zeros((128, WCST_COLS - 768), np.float32)
    mf = MSKF0 - 768
    for j in range(3):
        msk[:, mf + j * 128:mf + (j + 1) * 128] = tri128
    for j in range(3):
        msk[14 * j:14 * (j + 1), mf + 384 + 14 * j:mf + 384 + 14 * (j + 1)] \
            = tri14
    mt = MSKT0 - 768
    msk[:, mt:mt + 128] = tri128
    msk[0:14, mt + 128:mt + 142] = tri14
    return dict(wqkv=wqkv.astype(bf16), msk=msk.astype(bf16))


def _perm():
    # reordered token index -> natural (b*T + t) index, per core
    idx = []
    for g in range(NFULL):
        for j in range(GB):
            b = GB * g + j
            idx.append(np.arange(b * T, b * T + 128))
        for j in range(GB):
            b = GB * g + j
            idx.append(np.arange(b * T + 128, (b + 1) * T))
    b = NB - 1
    idx.append(np.arange(b * T, b * T + 128))
    idx.append(np.arange(b * T + 128, (b + 1) * T))
    return np.concatenate(idx)


_PERM = _perm()


def _prep_core_xt(x_core):
    # x_core [NB, T, C] fp32 -> [4, 128, NT] bf16 (x^T, reordered tokens)
    xall = x_core.reshape(NT, C)[_PERM]
    xtr = np.ascontiguousarray(xall.T).reshape(4, 128, NT)
    return xtr.astype(ml_dtypes.bfloat16)


def _assemble_core(om_np):
    # om [NG, 65, 468] f16 -> [NB, T, H] normalized fp32
    # tail-query columns hold two partial sums (main keys at 384:426,
    # tail keys at 426:468) that are added here
    om_np = om_np.astype(np.float32)
    full = om_np[:NFULL]                                 # [21, 65, 468]
    mains = full[:, :, 0:384].reshape(NFULL, TW, 3, 128)
    mains = mains.transpose(0, 2, 3, 1).reshape(63, 128, TW)
    tails = (full[:, :, 384:426] + full[:, :, 426:468]).reshape(
        NFULL, TW, 3, 14)
    tails = tails.transpose(0, 2, 3, 1).reshape(63, 14, TW)
    toks = np.concatenate([mains, tails], axis=1)        # [63, 142, 65]
    lt = om_np[NFULL, :, 128:142] + om_np[NFULL, :, 142:156]
    last = np.concatenate(
        [om_np[NFULL, :, 0:128].T, lt.T], axis=0)[None]  # [1, 142, 65]
    allb = np.concatenate([toks, last], axis=0)          # [64, 142, 65]
    return allb[:, :, 0:H] / allb[:, :, H:H + 1]


def kernel(**inputs):
    x = np.asarray(inputs["x"], dtype=np.float32)
    Wq = np.asarray(inputs["Wq"], dtype=np.float32)
    Wk = np.asarray(inputs["Wk"], dtype=np.float32)
    Wv = np.asarray(inputs["Wv"], dtype=np.float32)

    from concourse.bass_utils import run_bass_kernel_spmd

    if "nc" not in _CACHE:
        _CACHE["nc"] = _build_nc()
    nc = _CACHE["nc"]

    shared = _prep_shared(Wq, Wk, Wv)
    in_maps = []
    for core in range(NCORES):
        m = dict(shared)
        m["xt"] = _prep_core_xt(x[core * NB:(core + 1) * NB])
        in_maps.append(m)

    trace = bool(int(os.environ.get("TRN_KERNEL_TRACE", "0")))
    res = run_bass_kernel_spmd(
        nc, in_maps, core_ids=list(range(NCORES)), trace=trace,
    )
    _CACHE["last_result"] = res

    outs = []
    for core in range(NCORES):
        r = res.results[core]
        outs.append(_assemble_core(np.asarray(r["om"])))
    return np.concatenate(outs, axis=0).astype(np.float32)


# revision 83
# speedup vs baseline: 1.4453x; 1.0042x over previous
# Trainium2 Bass kernel for single-head causal attention
#   q = x@Wq, k = x@Wk, v = x@Wv   (x [B,T,C], W* [C,H])
#   out = softmax(mask(q k^T / sqrt(C))) @ v
# B=512, T=142, C=512, H=64.  Data-parallel over B across 8 NeuronCores.
#
# Per-core layout (64 batches, 21 groups of 3 + 1 tail group):
#  - host reorders tokens per group: [b0 0:128 | b1 0:128 | b2 0:128 |
#    b0 128:142 | b1 128:142 | b2 128:142] so every PE operand is a
#    contiguous column range.
#  - [Wq|Wk] packed stationary -> one PSUM accumulation; qT rows 0:64,
#    kT rows 64:128; one full-width PSUM->SBUF copy yields q in place,
#    and a partition-shifted engine copy (alternating ACT/DVE by group
#    parity for balance) re-bases kT to partition 0 so the scores
#    operands share a base, as the compiler requires.
#  - v computed x-stationary directly in natural [token, h] layout;
#    the 3 batch tails share one 42-column stationary tile; all v tiles
#    leave PSUM in a single engine copy.
#  - causal mask applied as a 0/1 bf16 multiply on DVE after exp
#    (no mask matmuls on the PE); main-keys-x-tail-queries columns are
#    fully valid and skip the mask entirely.
#  - tail scores use all 3 batches' key-tails as one stationary; the
#    cross-batch terms are zeroed by the mask multiply, enabling ONE
#    fused tail-AV matmul per group, written to its own PSUM columns
#    (the host adds the main-key and tail-key partial sums).
#  - AV is v-stationary [v|1] (65 cols) -> out^T [65, t] with the
#    softmax denominator in row 64; division happens on host.
#  - software pipeline: scores lag 2 and AV lag 4 behind the
#    projections, so copy/exp/mask/DMA latency never stalls the PE.
import os

import numpy as np
import ml_dtypes

B, T, C, H = 512, 142, 512, 64
NCORES = 8
NB = B // NCORES            # 64 batches per core
NT = NB * T                 # 9088 tokens per core
GB = 3                      # batches per full group
NG = 22                     # 21 full groups + 1 single-batch group
NFULL = 21
SCALE = float(C) ** -0.5
TW = 65                     # out rows: H + denominator row
# wcst column layout (all bf16)
WQK0 = 0                    # 4 chunks of [Wq_c | Wk_c] at 128*c
WV0 = 512                   # 4 chunks of Wv_c at 512 + 64*c
MSKF0 = 768                 # full-group mask [128, 426]
MSKT0 = 1194                # tail-group mask [128, 142]
WCST_COLS = 1336

_CACHE = {}


def _group_info(g):
    if g < NFULL:
        return GB, g * (GB * T)
    return 1, NFULL * (GB * T)


def _build_nc():
    import concourse.bacc as bacc
    import concourse.mybir as mybir
    from concourse.tile import TileContext

    fp32 = mybir.dt.float32
    fp16 = mybir.dt.float16
    bf16 = mybir.dt.bfloat16
    Exp = mybir.ActivationFunctionType.Exp

    nc = bacc.Bacc(
        "TRN2",
        target_bir_lowering=False,
        debug=False,
        enable_asserts=False,
        num_devices=NCORES,
    )

    xt = nc.dram_tensor("xt", [4, 128, NT], bf16, kind="ExternalInput").ap()
    # weights and masks staged separately so the critical first transfer
    # (wqkv, SP queue) is small; masks follow on the ACT queue.
    wqkv = nc.dram_tensor("wqkv", [128, 768], bf16, kind="ExternalInput").ap()
    msk = nc.dram_tensor("msk", [128, WCST_COLS - 768], bf16,
                         kind="ExternalInput").ap()
    om = nc.dram_tensor("om", [NG, TW, 468], fp16,
                        kind="ExternalOutput").ap()

    with TileContext(nc) as tc:
        with (
            tc.tile_pool(name="const", bufs=1) as cpool,
            tc.tile_pool(name="xtp", bufs=6) as xpool,
            tc.tile_pool(name="work", bufs=3) as wpool,
            tc.tile_pool(name="psum", bufs=1, space="PSUM") as ppool,
        ):
            wcst_sb = cpool.tile([128, WCST_COLS], bf16)
            # SWDGE path: its descriptor generation (Pool) runs in parallel
            # with the HWDGE generation of the first x transfers; wqk first
            # (the only blocker of the first matmul), wv right behind
            nc.gpsimd.dma_start(out=wcst_sb[:, 0:512], in_=wqkv[:, 0:512])
            nc.gpsimd.dma_start(out=wcst_sb[:, 512:768], in_=wqkv[:, 512:768])

            def wqk_c(c):
                return wcst_sb[:, WQK0 + c * 128:WQK0 + (c + 1) * 128]

            def wv_c(c):
                return wcst_sb[:, WV0 + c * 64:WV0 + (c + 1) * 64]

            xt_tiles = {}
            qsb, ksb, psc_t, exp_t, expm_t, vex_t = {}, {}, {}, {}, {}, {}
            pout_t = {}

            def issue_x_dma(g):
                nb, t0 = _group_info(g)
                gt = nb * T
                xt_t = xpool.tile([128, 4 * GB * T], bf16, tag="xt")
                xt_tiles[g] = xt_t
                if g == 0:
                    # split in halves for the first group so PE starts early
                    # (each extra DMA costs 625ns of serialized HWDGE gen)
                    for h in range(2):
                        nc.sync.dma_start(
                            out=xt_t[:, 2 * h * gt:2 * (h + 1) * gt].rearrange(
                                "p (c t) -> p c t", c=2),
                            in_=xt[2 * h:2 * (h + 1), :, 0:gt].rearrange(
                                "c p t -> p c t"))
                else:
                    nc.sync.dma_start(
                        out=xt_t[:, 0:4 * gt].rearrange(
                            "p (c t) -> p c t", c=4),
                        in_=xt[:, :, t0:t0 + gt].rearrange("c p t -> p c t"))

            def xslice(g, c, lo, hi):
                gt = _group_info(g)[0] * T
                base = c * gt
                return xt_tiles[g][:, base + lo:base + hi]

            def emit_qk(g):
                nb, _ = _group_info(g)
                gt = nb * T
                pqk = ppool.tile([128, GB * T], fp32, tag="pqk", bufs=2)
                for c in range(4):
                    nc.tensor.matmul(
                        pqk[:, 0:gt], lhsT=wqk_c(c), rhs=xslice(g, c, 0, gt),
                        start=(c == 0), stop=(c == 3))
                # one full-width PSUM->SBUF copy; rows 0:64 are qT, usable
                # directly.  kT (rows 64:128) is re-based to partition 0 by
                # a partition-shifted engine copy so the scores operands
                # share a base (a hard compiler requirement).
                qk_sb = wpool.tile([128, GB * T], bf16, tag="qk", bufs=4)
                k_sb = wpool.tile([64, GB * T], bf16, tag="k2", bufs=4)
                nc.scalar.copy(qk_sb[:, 0:gt], pqk[:, 0:gt])
                if 2 <= g < NG - 2:
                    # alternate the shifted re-base copy between engines
                    eng = nc.scalar.copy if g % 3 else nc.vector.tensor_copy
                    eng(k_sb[0:64, 0:gt], pqk[64:128, 0:gt])
                elif g < 2:
                    # warm-up: ACT is idle and the DMA round trip (+900ns
                    # sem) would sit on the critical path
                    nc.scalar.copy(k_sb[0:64, 0:gt], pqk[64:128, 0:gt])
                else:
                    # drain: keep the re-base on ACT, off the DVE o-chain
                    nc.scalar.copy(k_sb[0:64, 0:gt], pqk[64:128, 0:gt])
                qsb[g], ksb[g] = qk_sb, k_sb

            def emit_v(g):
                nb, _ = _group_info(g)
                pvt = ppool.tile([128, 256], fp32, tag="pvt", bufs=3)
                for j in range(nb):
                    for c in range(4):
                        nc.tensor.matmul(
                            pvt[:, j * 64:(j + 1) * 64],
                            lhsT=xslice(g, c, j * 128, (j + 1) * 128),
                            rhs=wv_c(c), start=(c == 0), stop=(c == 3))
                for c in range(4):
                    nc.tensor.matmul(
                        pvt[0:nb * 14, nb * 64:nb * 64 + 64],
                        lhsT=xslice(g, c, nb * 128, nb * 128 + nb * 14),
                        rhs=wv_c(c), start=(c == 0), stop=(c == 3))
                # one copy moves main v tiles AND the packed tail tile
                # (rows 42:126 of the tail block are dead but harmless)
                vex = wpool.tile([128, 4 * TW], bf16, tag="vex", bufs=6)
                nbl = nb + 1
                nc.vector.tensor_copy(
                    vex.rearrange("p (b h) -> p b h", h=TW)[:, 0:nbl, 0:64],
                    pvt[:, 0:nbl * 64].rearrange("p (b h) -> p b h", h=64),
                )
                if g < 6:  # ones columns: written once per pool buffer
                    nc.vector.memset(
                        vex.rearrange("p (b h) -> p b h", h=TW)[:, :, 64:65],
                        1.0)
                vex_t[g] = vex

            def emit_scores(g):
                # psc columns: [main nb*128 | sctt nb*14 | m2 nb*14]
                nb, _ = _group_info(g)
                MAIN = nb * 128
                S0, M0 = MAIN, MAIN + nb * 14
                psc = ppool.tile([128, 468], fp32, tag="psc", bufs=2)
                psc_t[g] = psc
                if g < 2:
                    # NaN-proof the never-written sctt rows once per buffer
                    # (engine partition windows: base 32 allows <=32 rows,
                    # base 64 allows <=64; rows 32:42 are rewritten by the
                    # tail-score matmuls right after)
                    nc.vector.memset(psc[32:64, S0:M0], 0.0)
                    nc.vector.memset(psc[64:128, S0:M0], 0.0)
                q_sb, k_sb = qsb.pop(g), ksb.pop(g)
                for j in range(nb):
                    nc.tensor.matmul(
                        psc[:, j * 128:(j + 1) * 128],
                        lhsT=k_sb[0:64, j * 128:(j + 1) * 128],
                        rhs=q_sb[0:64, j * 128:(j + 1) * 128],
                        start=True, stop=True)
                    nc.tensor.matmul(
                        psc[:, M0 + j * 14:M0 + (j + 1) * 14],
                        lhsT=k_sb[0:64, j * 128:(j + 1) * 128],
                        rhs=q_sb[0:64, MAIN + j * 14:MAIN + (j + 1) * 14],
                        start=True, stop=True)
                    nc.tensor.matmul(
                        psc[0:nb * 14, S0 + j * 14:S0 + (j + 1) * 14],
                        lhsT=k_sb[0:64, MAIN:MAIN + nb * 14],
                        rhs=q_sb[0:64, MAIN + j * 14:MAIN + (j + 1) * 14],
                        start=True, stop=True)

            def emit_exp(g):
                nb, _ = _group_info(g)
                expc = nb * 156
                mw = nb * 142        # masked width: main + sctt
                psc = psc_t.pop(g)
                exp_sb = wpool.tile([128, 468], bf16, tag="exp", bufs=4)
                expm = wpool.tile([128, 426], bf16, tag="expm", bufs=4)
                nc.scalar.activation(
                    exp_sb[:, 0:expc], psc[:, 0:expc], Exp, scale=SCALE)
                mo = MSKF0 if nb == GB else MSKT0
                nc.vector.tensor_mul(
                    expm[:, 0:mw], exp_sb[:, 0:mw], wcst_sb[:, mo:mo + mw])
                exp_t[g], expm_t[g] = exp_sb, expm

            def emit_av(g):
                nb, _ = _group_info(g)
                MAIN = nb * 128
                S0, M0 = MAIN, MAIN + nb * 14
                exp_sb, expm = exp_t.pop(g), expm_t.pop(g)
                vex = vex_t.pop(g)
                if g == NG - 1:
                    # psc banks are idle during the drain; a fresh tile
                    # avoids the pout WAR against o-copy(g-1)
                    pout = ppool.tile([128, 468], fp32, tag="psc", bufs=2)
                else:
                    pout = ppool.tile([TW, 468], fp32, tag="pout", bufs=1)
                pout_t[g] = pout
                for j in range(nb):
                    nc.tensor.matmul(
                        pout[0:TW, j * 128:(j + 1) * 128],
                        lhsT=vex[:, j * TW:(j + 1) * TW],
                        rhs=expm[0:128, j * 128:(j + 1) * 128],
                        start=True, stop=True)
                for j in range(nb):
                    nc.tensor.matmul(
                        pout[0:TW, MAIN + j * 14:MAIN + (j + 1) * 14],
                        lhsT=vex[:, j * TW:(j + 1) * TW],
                        rhs=exp_sb[0:128, M0 + j * 14:M0 + (j + 1) * 14],
                        start=True, stop=True)
                # tail-key contributions land in their own columns
                # (M0:M0+nb*14); the host adds the two partial sums
                nc.tensor.matmul(
                    pout[0:TW, M0:M0 + nb * 14],
                    lhsT=vex[0:nb * 14, nb * TW:(nb + 1) * TW],
                    rhs=expm[0:nb * 14, S0:M0],
                    start=True, stop=True)

            def emit_out(g):
                nb, _ = _group_info(g)
                ow = nb * 156
                pout = pout_t.pop(g)
                o_sb = wpool.tile([TW, 468], fp16, tag="o", bufs=4)
                nc.vector.tensor_copy(o_sb[0:TW, 0:ow], pout[0:TW, 0:ow])
                eng = nc.sync if g >= NG - 5 else nc.gpsimd
                eng.dma_start(out=om[g, :, 0:ow], in_=o_sb[0:TW, 0:ow])

            # software pipeline: iteration i runs
            #   qk(i), AV(i-4), out(i-4), scores(i-2), exp/mask(i-2), v(i)
            # with compressed lags over the last groups (drain phase: the
            # PE is idle there, so latency-hiding lags only stretch the
            # tail)
            sc_at = {g: g + 2 for g in range(NG)}
            av_at = {g: g + 4 for g in range(NG)}
            last_i = max(av_at.values())
            for g in range(3):
                issue_x_dma(g)
            for i in range(last_i + 1):
                if i + 3 < NG:
                    issue_x_dma(i + 3)
                if i == 1:
                    # masks are first read by mask(0) at iteration 2;
                    # deferring this transfer keeps the early DMA queue
                    # clear for x chunks and the first k re-base DMAs
                    nc.scalar.dma_start(
                        out=wcst_sb[:, 768:WCST_COLS], in_=msk)
                late = i >= NG - 4
                if late:
                    # drain: exp chains gate the remaining AVs; emit them
                    # ahead of the qk copies, which have slack
                    for g in range(NG):
                        if sc_at[g] == i:
                            emit_scores(g)
                            emit_exp(g)
                if i < NG:
                    emit_qk(i)
                for g in range(NG):
                    if av_at[g] == i:
                        emit_av(g)
                        emit_out(g)
                if not late:
                    for g in range(NG):
                        if sc_at[g] == i:
                            emit_scores(g)
                            emit_exp(g)
                if i < NG:
                    emit_v(i)

    nc.compile()
    return nc


def _prep_shared(Wq, Wk, Wv):
    bf16 = ml_dtypes.bfloat16
    wqkv = np.zeros((128, 768), np.float32)
    for c in range(4):
        wqkv[:, WQK0 + c * 128:WQK0 + c * 128 + 64] = Wq[c * 128:(c + 1) * 128]
        wqkv[:, WQK0 + c * 128 + 64:WQK0 + (c + 1) * 128] = \
            Wk[c * 128:(c + 1) * 128]
        wqkv[:, WV0 + c * 64:WV0 + (c + 1) * 64] = Wv[c * 128:(c + 1) * 128]

    s = np.arange(128)[:, None]
    t = np.arange(128)[None, :]
    tri128 = (s <= t).astype(np.float32)          # valid (unmasked) = 1
    s14 = np.arange(14)[:, None]
    t14 = np.arange(14)[None, :]
    tri14 = (s14 <= t14).astype(np.float32)

    msk = np.zeros((128, WCST_COLS - 768), np.float32)
    mf = MSKF0 - 768
    for j in range(3):
        msk[:, mf + j * 128:mf + (j + 1) * 128] = tri128
    for j in range(3):
        msk[14 * j:14 * (j + 1), mf + 384 + 14 * j:mf + 384 + 14 * (j + 1)] \
            = tri14
    mt = MSKT0 - 768
    msk[:, mt:mt + 128] = tri128
    msk[0:14, mt + 128:mt + 142] = tri14
    return dict(wqkv=wqkv.astype(bf16), msk=msk.astype(bf16))


def _perm():
    # reordered token index -> natural (b*T + t) index, per core
    idx = []
    for g in range(NFULL):
        for j in range(GB):
            b = GB * g + j
            idx.append(np.arange(b * T, b * T + 128))
        for j in range(GB):
            b = GB * g + j
            idx.append(np.arange(b * T + 128, (b + 1) * T))
    b = NB - 1
    idx.append(np.arange(b * T, b * T + 128))
    idx.append(np.arange(b * T + 128, (b + 1) * T))
    return np.concatenate(idx)


_PERM = _perm()


def _prep_core_xt(x_core):
    # x_core [NB, T, C] fp32 -> [4, 128, NT] bf16 (x^T, reordered tokens)
    xall = x_core.reshape(NT, C)[_PERM]
    xtr = np.ascontiguousarray(xall.T).reshape(4, 128, NT)
    return xtr.astype(ml_dtypes.bfloat16)


def _assemble_core(om_np):
    # om [NG, 65, 468] f16 -> [NB, T, H] normalized fp32
    # tail-query columns hold two partial sums (main keys at 384:426,
    # tail keys at 426:468) that are added here
    om_np = om_np.astype(np.float32)
    full = om_np[:NFULL]                                 # [21, 65, 468]
    mains = full[:, :, 0:384].reshape(NFULL, TW, 3, 128)
    mains = mains.transpose(0, 2, 3, 1).reshape(63, 128, TW)
    tails = (full[:, :, 384:426] + full[:, :, 426:468]).reshape(
        NFULL, TW, 3, 14)
    tails = tails.transpose(0, 2, 3, 1).reshape(63, 14, TW)
    toks = np.concatenate([mains, tails], axis=1)        # [63, 142, 65]
    lt = om_np[NFULL, :, 128:142] + om_np[NFULL, :, 142:156]
    last = np.concatenate(
        [om_np[NFULL, :, 0:128].T, lt.T], axis=0)[None]  # [1, 142, 65]
    allb = np.concatenate([toks, last], axis=0)          # [64, 142, 65]
    return allb[:, :, 0:H] / allb[:, :, H:H + 1]


def kernel(**inputs):
    x = np.asarray(inputs["x"], dtype=np.float32)
    Wq = np.asarray(inputs["Wq"], dtype=np.float32)
    Wk = np.asarray(inputs["Wk"], dtype=np.float32)
    Wv = np.asarray(inputs["Wv"], dtype=np.float32)

    from concourse.bass_utils import run_bass_kernel_spmd

    if "nc" not in _CACHE:
        _CACHE["nc"] = _build_nc()
    nc = _CACHE["nc"]

    shared = _prep_shared(Wq, Wk, Wv)
    in_maps = []
    for core in range(NCORES):
        m = dict(shared)
        m["xt"] = _prep_core_xt(x[core * NB:(core + 1) * NB])
        in_maps.append(m)

    trace = bool(int(os.environ.get("TRN_KERNEL_TRACE", "0")))
    res = run_bass_kernel_spmd(
        nc, in_maps, core_ids=list(range(NCORES)), trace=trace,
    )
    _CACHE["last_result"] = res

    outs = []
    for core in range(NCORES):
        r = res.results[core]
        outs.append(_assemble_core(np.asarray(r["om"])))
    return np.concatenate(outs, axis=0).astype(np.float32)


# revision 86
# speedup vs baseline: 1.4459x; 1.0004x over previous
# Trainium2 Bass kernel for single-head causal attention
#   q = x@Wq, k = x@Wk, v = x@Wv   (x [B,T,C], W* [C,H])
#   out = softmax(mask(q k^T / sqrt(C))) @ v
# B=512, T=142, C=512, H=64.  Data-parallel over B across 8 NeuronCores.
#
# Per-core layout (64 batches, 21 groups of 3 + 1 tail group):
#  - host reorders tokens per group: [b0 0:128 | b1 0:128 | b2 0:128 |
#    b0 128:142 | b1 128:142 | b2 128:142] so every PE operand is a
#    contiguous column range.
#  - [Wq|Wk] packed stationary -> one PSUM accumulation; qT rows 0:64,
#    kT rows 64:128; one full-width PSUM->SBUF copy yields q in place,
#    and a partition-shifted engine copy (alternating ACT/DVE by group
#    parity for balance) re-bases kT to partition 0 so the scores
#    operands share a base, as the compiler requires.
#  - v computed x-stationary directly in natural [token, h] layout;
#    the 3 batch tails share one 42-column stationary tile; all v tiles
#    leave PSUM in a single engine copy.
#  - causal mask applied as a 0/1 bf16 multiply on DVE after exp
#    (no mask matmuls on the PE); main-keys-x-tail-queries columns are
#    fully valid and skip the mask entirely.
#  - tail scores use all 3 batches' key-tails as one stationary; the
#    cross-batch terms are zeroed by the mask multiply, enabling ONE
#    fused tail-AV matmul per group, written to its own PSUM columns
#    (the host adds the main-key and tail-key partial sums).
#  - AV is v-stationary [v|1] (65 cols) -> out^T [65, t] with the
#    softmax denominator in row 64; division happens on host.
#  - software pipeline: scores lag 2 and AV lag 4 behind the
#    projections, so copy/exp/mask/DMA latency never stalls the PE.
import os

import numpy as np
import ml_dtypes

B, T, C, H = 512, 142, 512, 64
NCORES = 8
NB = B // NCORES            # 64 batches per core
NT = NB * T                 # 9088 tokens per core
GB = 3                      # batches per full group
NG = 22                     # 21 full groups + 1 single-batch group
NFULL = 21
SCALE = float(C) ** -0.5
TW = 65                     # out rows: H + denominator row
# wcst column layout (all bf16)
WQK0 = 0                    # 4 chunks of [Wq_c | Wk_c] at 128*c
WV0 = 512                   # 4 chunks of Wv_c at 512 + 64*c
MSKF0 = 768                 # full-group mask [128, 426]
MSKT0 = 1194                # tail-group mask [128, 142]
WCST_COLS = 1336

_CACHE = {}


def _group_info(g):
    if g < NFULL:
        return GB, g * (GB * T)
    return 1, NFULL * (GB * T)


def _build_nc():
    import concourse.bacc as bacc
    import concourse.mybir as mybir
    from concourse.tile import TileContext

    fp32 = mybir.dt.float32
    fp16 = mybir.dt.float16
    bf16 = mybir.dt.bfloat16
    Exp = mybir.ActivationFunctionType.Exp

    nc = bacc.Bacc(
        "TRN2",
        target_bir_lowering=False,
        debug=False,
        enable_asserts=False,
        num_devices=NCORES,
    )

    xt = nc.dram_tensor("xt", [4, 128, NT], bf16, kind="ExternalInput").ap()
    # weights and masks staged separately so the critical first transfer
    # (wqkv, SP queue) is small; masks follow on the ACT queue.
    wqkv = nc.dram_tensor("wqkv", [128, 768], bf16, kind="ExternalInput").ap()
    msk = nc.dram_tensor("msk", [128, WCST_COLS - 768], bf16,
                         kind="ExternalInput").ap()
    om = nc.dram_tensor("om", [NG, TW, 468], fp16,
                        kind="ExternalOutput").ap()

    with TileContext(nc) as tc:
        with (
            tc.tile_pool(name="const", bufs=1) as cpool,
            tc.tile_pool(name="xtp", bufs=6) as xpool,
            tc.tile_pool(name="work", bufs=3) as wpool,
            tc.tile_pool(name="psum", bufs=1, space="PSUM") as ppool,
        ):
            wcst_sb = cpool.tile([128, WCST_COLS], bf16)
            # SWDGE path: its descriptor generation (Pool) runs in parallel
            # with the HWDGE generation of the first x transfers; wqk first
            # (the only blocker of the first matmul), wv right behind
            nc.gpsimd.dma_start(out=wcst_sb[:, 0:512], in_=wqkv[:, 0:512])
            nc.gpsimd.dma_start(out=wcst_sb[:, 512:768], in_=wqkv[:, 512:768])

            def wqk_c(c):
                return wcst_sb[:, WQK0 + c * 128:WQK0 + (c + 1) * 128]

            def wv_c(c):
                return wcst_sb[:, WV0 + c * 64:WV0 + (c + 1) * 64]

            xt_tiles = {}
            qsb, ksb, psc_t, exp_t, expm_t, vex_t = {}, {}, {}, {}, {}, {}
            pout_t = {}

            def issue_x_dma(g):
                nb, t0 = _group_info(g)
                gt = nb * T
                xt_t = xpool.tile([128, 4 * GB * T], bf16, tag="xt")
                xt_tiles[g] = xt_t
                if g == 0:
                    # split in halves for the first group so PE starts early
                    # (each extra DMA costs 625ns of serialized HWDGE gen)
                    for h in range(2):
                        nc.sync.dma_start(
                            out=xt_t[:, 2 * h * gt:2 * (h + 1) * gt].rearrange(
                                "p (c t) -> p c t", c=2),
                            in_=xt[2 * h:2 * (h + 1), :, 0:gt].rearrange(
                                "c p t -> p c t"))
                else:
                    nc.sync.dma_start(
                        out=xt_t[:, 0:4 * gt].rearrange(
                            "p (c t) -> p c t", c=4),
                        in_=xt[:, :, t0:t0 + gt].rearrange("c p t -> p c t"))

            def xslice(g, c, lo, hi):
                gt = _group_info(g)[0] * T
                base = c * gt
                return xt_tiles[g][:, base + lo:base + hi]

            def emit_qk(g):
                nb, _ = _group_info(g)
                gt = nb * T
                pqk = ppool.tile([128, GB * T], fp32, tag="pqk", bufs=2)
                for c in range(4):
                    nc.tensor.matmul(
                        pqk[:, 0:gt], lhsT=wqk_c(c), rhs=xslice(g, c, 0, gt),
                        start=(c == 0), stop=(c == 3))
                # one full-width PSUM->SBUF copy; rows 0:64 are qT, usable
                # directly.  kT (rows 64:128) is re-based to partition 0 by
                # a partition-shifted engine copy so the scores operands
                # share a base (a hard compiler requirement).
                qk_sb = wpool.tile([128, GB * T], bf16, tag="qk", bufs=4)
                k_sb = wpool.tile([64, GB * T], bf16, tag="k2", bufs=4)
                nc.scalar.copy(qk_sb[:, 0:gt], pqk[:, 0:gt])
                if 2 <= g < NG - 2:
                    # alternate the shifted re-base copy between engines
                    eng = nc.scalar.copy if g % 3 else nc.vector.tensor_copy
                    eng(k_sb[0:64, 0:gt], pqk[64:128, 0:gt])
                elif g < 2:
                    # warm-up: ACT is idle and the DMA round trip (+900ns
                    # sem) would sit on the critical path
                    nc.scalar.copy(k_sb[0:64, 0:gt], pqk[64:128, 0:gt])
                else:
                    # drain: keep the re-base on ACT, off the DVE o-chain
                    nc.scalar.copy(k_sb[0:64, 0:gt], pqk[64:128, 0:gt])
                qsb[g], ksb[g] = qk_sb, k_sb

            def emit_v(g):
                nb, _ = _group_info(g)
                pvt = ppool.tile([128, 256], fp32, tag="pvt", bufs=3)
                for j in range(nb):
                    for c in range(4):
                        nc.tensor.matmul(
                            pvt[:, j * 64:(j + 1) * 64],
                            lhsT=xslice(g, c, j * 128, (j + 1) * 128),
                            rhs=wv_c(c), start=(c == 0), stop=(c == 3))
                for c in range(4):
                    nc.tensor.matmul(
                        pvt[0:nb * 14, nb * 64:nb * 64 + 64],
                        lhsT=xslice(g, c, nb * 128, nb * 128 + nb * 14),
                        rhs=wv_c(c), start=(c == 0), stop=(c == 3))
                # one copy moves main v tiles AND the packed tail tile
                # (rows 42:126 of the tail block are dead but harmless)
                vex = wpool.tile([128, 4 * TW], bf16, tag="vex", bufs=6)
                nbl = nb + 1
                nc.vector.tensor_copy(
                    vex.rearrange("p (b h) -> p b h", h=TW)[:, 0:nbl, 0:64],
                    pvt[:, 0:nbl * 64].rearrange("p (b h) -> p b h", h=64),
                )
                if g < 6:  # ones columns: written once per pool buffer
                    nc.vector.memset(
                        vex.rearrange("p (b h) -> p b h", h=TW)[:, :, 64:65],
                        1.0)
                vex_t[g] = vex

            def emit_scores(g):
                # psc columns: [main nb*128 | sctt nb*14 | m2 nb*14]
                nb, _ = _group_info(g)
                MAIN = nb * 128
                S0, M0 = MAIN, MAIN + nb * 14
                psc = ppool.tile([128, 468], fp32, tag="psc", bufs=2)
                psc_t[g] = psc
                if g < 2:
                    # NaN-proof the never-written sctt rows once per buffer
                    # (engine partition windows: base 32 allows <=32 rows,
                    # base 64 allows <=64; rows 32:42 are rewritten by the
                    # tail-score matmuls right after)
                    nc.vector.memset(psc[32:64, S0:M0], 0.0)
                    nc.vector.memset(psc[64:128, S0:M0], 0.0)
                q_sb, k_sb = qsb.pop(g), ksb.pop(g)
                for j in range(nb):
                    nc.tensor.matmul(
                        psc[:, j * 128:(j + 1) * 128],
                        lhsT=k_sb[0:64, j * 128:(j + 1) * 128],
                        rhs=q_sb[0:64, j * 128:(j + 1) * 128],
                        start=True, stop=True)
                    nc.tensor.matmul(
                        psc[:, M0 + j * 14:M0 + (j + 1) * 14],
                        lhsT=k_sb[0:64, j * 128:(j + 1) * 128],
                        rhs=q_sb[0:64, MAIN + j * 14:MAIN + (j + 1) * 14],
                        start=True, stop=True)
                    nc.tensor.matmul(
                        psc[0:nb * 14, S0 + j * 14:S0 + (j + 1) * 14],
                        lhsT=k_sb[0:64, MAIN:MAIN + nb * 14],
                        rhs=q_sb[0:64, MAIN + j * 14:MAIN + (j + 1) * 14],
                        start=True, stop=True)

            def emit_exp(g):
                nb, _ = _group_info(g)
                expc = nb * 156
                mw = nb * 142        # masked width: main + sctt
                psc = psc_t.pop(g)
                exp_sb = wpool.tile([128, 468], bf16, tag="exp", bufs=4)
                expm = wpool.tile([128, 426], bf16, tag="expm", bufs=4)
                nc.scalar.activation(
                    exp_sb[:, 0:expc], psc[:, 0:expc], Exp, scale=SCALE)
                mo = MSKF0 if nb == GB else MSKT0
                nc.vector.tensor_mul(
                    expm[:, 0:mw], exp_sb[:, 0:mw], wcst_sb[:, mo:mo + mw])
                exp_t[g], expm_t[g] = exp_sb, expm

            def emit_av(g):
                nb, _ = _group_info(g)
                MAIN = nb * 128
                S0, M0 = MAIN, MAIN + nb * 14
                exp_sb, expm = exp_t.pop(g), expm_t.pop(g)
                vex = vex_t.pop(g)
                if g == NG - 1:
                    # psc banks are idle during the drain; a fresh tile
                    # avoids the pout WAR against o-copy(g-1)
                    pout = ppool.tile([128, 468], fp32, tag="psc", bufs=2)
                else:
                    pout = ppool.tile([TW, 468], fp32, tag="pout", bufs=1)
                pout_t[g] = pout
                for j in range(nb):
                    nc.tensor.matmul(
                        pout[0:TW, j * 128:(j + 1) * 128],
                        lhsT=vex[:, j * TW:(j + 1) * TW],
                        rhs=expm[0:128, j * 128:(j + 1) * 128],
                        start=True, stop=True)
                for j in range(nb):
                    nc.tensor.matmul(
                        pout[0:TW, MAIN + j * 14:MAIN + (j + 1) * 14],
                        lhsT=vex[:, j * TW:(j + 1) * TW],
                        rhs=exp_sb[0:128, M0 + j * 14:M0 + (j + 1) * 14],
                        start=True, stop=True)
                # tail-key contributions land in their own columns
                # (M0:M0+nb*14); the host adds the two partial sums
                nc.tensor.matmul(
                    pout[0:TW, M0:M0 + nb * 14],
                    lhsT=vex[0:nb * 14, nb * TW:(nb + 1) * TW],
                    rhs=expm[0:nb * 14, S0:M0],
                    start=True, stop=True)

            def emit_out(g):
                nb, _ = _group_info(g)
                ow = nb * 156
                pout = pout_t.pop(g)
                o_sb = wpool.tile([TW, 468], fp16, tag="o", bufs=4)
                nc.vector.tensor_copy(o_sb[0:TW, 0:ow], pout[0:TW, 0:ow])
                eng = nc.sync if g >= NG - 5 else nc.gpsimd
                eng.dma_start(out=om[g, :, 0:ow], in_=o_sb[0:TW, 0:ow])

            # software pipeline: iteration i runs
            #   qk(i), AV(i-4), out(i-4), scores(i-2), exp/mask(i-2), v(i)
            # with compressed lags over the last groups (drain phase: the
            # PE is idle there, so latency-hiding lags only stretch the
            # tail)
            sc_at = {g: g + 2 for g in range(NG)}
            av_at = {g: g + 4 for g in range(NG)}
            last_i = max(av_at.values())
            for g in range(3):
                issue_x_dma(g)
            for i in range(last_i + 1):
                if i + 3 < NG:
                    issue_x_dma(i + 3)
                if i == 1:
                    # masks are first read by mask(0) at iteration 2;
                    # deferring this transfer keeps the early DMA queue
                    # clear for x chunks and the first k re-base DMAs
                    nc.scalar.dma_start(
                        out=wcst_sb[:, 768:WCST_COLS], in_=msk)
                late = i >= NG - 2
                if late:
                    # drain: exp chains gate the remaining AVs; emit them
                    # ahead of the qk copies, which have slack
                    for g in range(NG):
                        if sc_at[g] == i:
                            emit_scores(g)
                            emit_exp(g)
                if i < NG:
                    emit_qk(i)
                for g in range(NG):
                    if av_at[g] == i:
                        emit_av(g)
                        emit_out(g)
                if not late:
                    for g in range(NG):
                        if sc_at[g] == i:
                            emit_scores(g)
                            emit_exp(g)
                if i < NG:
                    emit_v(i)

    nc.compile()
    return nc


def _prep_shared(Wq, Wk, Wv):
    bf16 = ml_dtypes.bfloat16
    wqkv = np.zeros((128, 768), np.float32)
    for c in range(4):
        wqkv[:, WQK0 + c * 128:WQK0 + c * 128 + 64] = Wq[c * 128:(c + 1) * 128]
        wqkv[:, WQK0 + c * 128 + 64:WQK0 + (c + 1) * 128] = \
            Wk[c * 128:(c + 1) * 128]
        wqkv[:, WV0 + c * 64:WV0 + (c + 1) * 64] = Wv[c * 128:(c + 1) * 128]

    s = np.arange(128)[:, None]
    t = np.arange(128)[None, :]
    tri128 = (s <= t).astype(np.float32)          # valid (unmasked) = 1
    s14 = np.arange(14)[:, None]
    t14 = np.arange(14)[None, :]
    tri14 = (s14 <= t14).astype(np.float32)

    msk = np.zeros((128, WCST_COLS - 768), np.float32)
    mf = MSKF0 - 768
    for j in range(3):
        msk[:, mf + j * 128:mf + (j + 1) * 128] = tri128
    for j in range(3):
        msk[14 * j:14 * (j + 1), mf + 384 + 14 * j:mf + 384 + 14 * (j + 1)] \
            = tri14
    mt = MSKT0 - 768
    msk[:, mt:mt + 128] = tri128
    msk[0:14, mt + 128:mt + 142] = tri14
    return dict(wqkv=wqkv.astype(bf16), msk=msk.astype(bf16))


def _perm():
    # reordered token index -> natural (b*T + t) index, per core
    idx = []
    for g in range(NFULL):
        for j in range(GB):
            b = GB * g + j
            idx.append(np.arange(b * T, b * T + 128))
        for j in range(GB):
            b = GB * g + j
            idx.append(np.arange(b * T + 128, (b + 1) * T))
    b = NB - 1
    idx.append(np.arange(b * T, b * T + 128))
    idx.append(np.arange(b * T + 128, (b + 1) * T))
    return np.concatenate(idx)


_PERM = _perm()


def _prep_core_xt(x_core):
    # x_core [NB, T, C] fp32 -> [4, 128, NT] bf16 (x^T, reordered tokens)
    xall = x_core.reshape(NT, C)[_PERM]
    xtr = np.ascontiguousarray(xall.T).reshape(4, 128, NT)
    return xtr.astype(ml_dtypes.bfloat16)


def _assemble_core(om_np):
    # om [NG, 65, 468] f16 -> [NB, T, H] normalized fp32
    # tail-query columns hold two partial sums (main keys at 384:426,
    # tail keys at 426:468) that are added here
    om_np = om_np.astype(np.float32)
    full = om_np[:NFULL]                                 # [21, 65, 468]
    mains = full[:, :, 0:384].reshape(NFULL, TW, 3, 128)
    mains = mains.transpose(0, 2, 3, 1).reshape(63, 128, TW)
    tails = (full[:, :, 384:426] + full[:, :, 426:468]).reshape(
        NFULL, TW, 3, 14)
    tails = tails.transpose(0, 2, 3, 1).reshape(63, 14, TW)
    toks = np.concatenate([mains, tails], axis=1)        # [63, 142, 65]
    lt = om_np[NFULL, :, 128:142] + om_np[NFULL, :, 142:156]
    last = np.concatenate(
        [om_np[NFULL, :, 0:128].T, lt.T], axis=0)[None]  # [1, 142, 65]
    allb = np.concatenate([toks, last], axis=0)          # [64, 142, 65]
    return allb[:, :, 0:H] / allb[:, :, H:H + 1]


def kernel(**inputs):
    x = np.asarray(inputs["x"], dtype=np.float32)
    Wq = np.asarray(inputs["Wq"], dtype=np.float32)
    Wk = np.asarray(inputs["Wk"], dtype=np.float32)
    Wv = np.asarray(inputs["Wv"], dtype=np.float32)

    from concourse.bass_utils import run_bass_kernel_spmd

    if "nc" not in _CACHE:
        _CACHE["nc"] = _build_nc()
    nc = _CACHE["nc"]

    shared = _prep_shared(Wq, Wk, Wv)
    in_maps = []
    for core in range(NCORES):
        m = dict(shared)
        m["xt"] = _prep_core_xt(x[core * NB:(core + 1) * NB])
        in_maps.append(m)

    trace = bool(int(os.environ.get("TRN_KERNEL_TRACE", "0")))
    res = run_bass_kernel_spmd(
        nc, in_maps, core_ids=list(range(NCORES)), trace=trace,
    )
    _CACHE["last_result"] = res

    outs = []
    for core in range(NCORES):
        r = res.results[core]
        outs.append(_assemble_core(np.asarray(r["om"])))
    return np.concatenate(outs, axis=0).astype(np.float32)
